# revision 1
# baseline (speedup 1.0000x reference)
"""CapsNet routing-by-agreement kernel for 8 TRN2 NeuronCores.

Strategy (in_caps sharded 8-way):
  - Each core owns I_loc = 512 in_caps. Its W shard lives entirely in SBUF
    (two layouts), so routing iterations do ZERO HBM traffic for W / u_hat.
    u_hat is never materialized; each routing iteration recomputes the two
    W contractions on the TensorEngine with 128-deep packed contractions:
      a-path:  Wv[b,i,o,k] = sum_j W[i,o,j,k] v[b,o,j]   (contract (o8,j)=128,
               block-diag v as stationary operand)
               a[b,i,o]    = sum_k u[b,i,k] Wv[b,i,o,k]  (DVE mul + add-tree)
      s-path:  s[b,o,j]    = sum_{i,k} (c*u)[..] W[..]   (contract i mod 128,
               PSUM-accum over (i-block, k); 8x block-diag fanout over o8,
               diagonal extracted with a constant mask)
  - The only cross-core data is the per-out-capsule sum s [64,32,16]:
    AllGather (fp8 for routing iterations, bf16 for the final one) + local
    on-chip sum, once per routing iteration (3x).
  - Elementwise work is load-balanced across DVE / ACT / Pool(GPSIMD) via
    the PLAN knobs; PSUM drains rotate over all three engines.
  - All layout permutations of the inputs are done host-side in numpy; every
    on-chip tensor is DMA'd contiguously.

Index conventions (per core c): i_glob = c*512 + Gq*128 + p (Gq in 0..3,
p in 0..127);  o = oc*8 + o8 (oc in 0..3);  q = o8*16 + j (j in 0..15).

Host layouts:
  wd  [128,16384] : wd[q, ((oc*4+Gq)*8+k)*128 + p]  = W[i,o,j,k] * WS
  wb  [128,16384] : wb[p, ((Gq*8+k)*4+oc)*128 + q]  = W[i,o,j,k] * WS
  uy  [128, 2048] bf16 : uy[p, (Gq*8+k)*64 + b]     = u[b,i,k]
  dlt [128,  128] bf16 : dlt[q, m] = (q//16 == m//16)  (squash sum_j)
  mbd [128, 2048] bf16 : mbd[q, (oc,o8p,b)] = (q//16 == o8p)
  msk [128,  512] f32  : msk[q, (o8p,b)]   = (q//16 == o8p) / WS
Output:
  vout [128, 256] f32 : vout[o8*16+j, oc*64+b] = v[b, oc*8+o8, j]
"""

import json
import os
import sys

import numpy as np
import ml_dtypes

sys.path.insert(0, "/opt/trn_rl_repo")

B, IN_CAPS, IN_DIM = 64, 4096, 8
OUT_CAPS, OUT_DIM = 32, 16
EPS = 1e-8
N_CORES = 8
I_LOC = IN_CAPS // N_CORES  # 512
NG = I_LOC // 128           # 4

_BF16 = ml_dtypes.bfloat16

# Engine-assignment plan. Letters: A=ACT(scalar), D=DVE(vector), P=Pool(gpsimd)
PLAN = {
    # one char per wv-phase drain chunk, idx = k*2 + ocp (16 chunks of
    # [128,1024] f32 PSUM -> bf16 SBUF per Gq). Pool/GPSIMD cannot read
    # PSUM (BIR verifier), so only A/D are valid here.
    "drain": "AAAAAAAAAAAAAAAA",
    # which k's (0..7) of the cu build (c*u) go to Pool
    "cu_pool_ks": (5, 6, 7),
    # which k's of the a-path mul (Wv*u) go to Pool
    "amul_pool_ks": (),
    # squash via fused tensor_scalar pow (vs Ln/Exp on ACT)
    "sq_pow": True,
    # fp8 payload for the routing AllGathers (iterations 1, 2).
    # Measured on HW: costs ~1.3e-2 of rel err — too close to the 2e-2
    # gate to enable by default.
    "ag_fp8": False,
    # W layouts in fp8e4 (scaled by WS), stationary-only. Measured on HW:
    # ~2.4e-2 rel err through routing feedback — keep off.
    "wfp8": False,
}
if os.environ.get("CAPS_PLAN"):
    PLAN.update(json.loads(os.environ["CAPS_PLAN"]))

WS = 16.0  # host-side W scale for fp8 dynamic range (descaled on chip)

_CACHE = {}


def _build_program(reps=1, variant="full", plan=None):
    import concourse.bass as bass
    import concourse.bacc as bacc
    import concourse.mybir as mybir
    import concourse.tile as tile
    from contextlib import ExitStack

    p = dict(PLAN)
    if plan:
        p.update(plan)

    f32 = mybir.dt.float32
    bf16 = mybir.dt.bfloat16
    fp8 = mybir.dt.float8e4
    wdt = fp8 if p["wfp8"] else bf16
    wscale = WS if p["wfp8"] else 1.0
    vset = set(variant.split(","))
    exch = ("ag" if "ag" in vset else
            "ar" if "ar" in vset else
            "noexch" if "noexch" in vset else "ag")
    AF = mybir.ActivationFunctionType
    ALU = mybir.AluOpType

    nc = bacc.Bacc(
        "TRN2",
        target_bir_lowering=False,
        debug=False,
        enable_asserts=False,
        num_devices=N_CORES,
    )

    wd_d = nc.dram_tensor("wd", [128, 16384], wdt, kind="ExternalInput")
    wb_d = nc.dram_tensor("wb", [128, 16384], wdt, kind="ExternalInput")
    uy_d = nc.dram_tensor("uy", [128, 2048], bf16, kind="ExternalInput")
    dlt_d = nc.dram_tensor("dlt", [128, 128], bf16, kind="ExternalInput")
    mbd_d = nc.dram_tensor("mbd", [128, 2048], bf16, kind="ExternalInput")
    msk_d = nc.dram_tensor("msk", [128, 512], f32, kind="ExternalInput")
    vout_d = nc.dram_tensor("vout", [128, 256], f32, kind="ExternalOutput")

    with tile.TileContext(nc) as tc:
        with ExitStack() as ctx:
            sb = ctx.enter_context(tc.tile_pool(name="sb", bufs=1))
            ps_pool_s = ctx.enter_context(
                tc.tile_pool(name="ps_s", bufs=1, space="PSUM"))
            ps_pool_wv = ctx.enter_context(
                tc.tile_pool(name="ps_wv", bufs=2, space="PSUM"))
            dram = ctx.enter_context(tc.tile_pool(name="dram", bufs=2,
                                                  space="DRAM"))

            WD = sb.tile([128, 16384], wdt, tag="WD")
            WBH = [sb.tile([128, 8192], wdt, tag=f"WB{i}", name=f"WB{i}")
                   for i in range(2)]
            UY = sb.tile([128, 2048], bf16, tag="UY")
            DLT = sb.tile([128, 128], bf16, tag="DLT")
            MBD = sb.tile([128, 2048], bf16, tag="MBD")
            MSK = sb.tile([128, 512], f32, tag="MSK")

            # DMA order: iteration-1 needs WB halves + UY first; WD (a-path)
            # is only needed after the first AllGather; constants in between.
            nc.sync.dma_start(WBH[0][:], wb_d[:, 0:8192])
            nc.sync.dma_start(UY[:, 0:1024], uy_d[:, 0:1024])
            nc.sync.dma_start(WBH[1][:], wb_d[:, 8192:16384])
            nc.sync.dma_start(UY[:, 1024:2048], uy_d[:, 1024:2048])
            nc.sync.dma_start(DLT[:], dlt_d[:])
            nc.sync.dma_start(MBD[:], mbd_d[:])
            nc.sync.dma_start(MSK[:], msk_d[:])
            nc.sync.dma_start(WD[:], wd_d[:])

            def WBs(idx):
                half, off = divmod(idx * 128, 8192)
                return WBH[half][:, off : off + 128]

            # big per-G scratch, parity double-buffered:
            # wv (drains) -> uwv (in-place mul) -> tree partials -> cu
            WVP = [sb.tile([128, 16384], bf16, tag=f"WV{i}", name=f"WV{i}")
                   for i in range(2)]
            BLOG = sb.tile([128, 8192], bf16, tag="BLOG")
            AG = sb.tile([128, 2048], bf16, tag="AG")
            EG = sb.tile([128, 2048], bf16, tag="EG")
            URC = sb.tile([128, 512], bf16, tag="URC")
            SMT = sb.tile([128, 1024], bf16, tag="SMT")
            MSKD = sb.tile([128, 512], f32, tag="MSKD")
            GAT = sb.tile([128, 2048], bf16, tag="GAT")
            GAT8 = sb.tile([128, 2048], fp8, tag="GAT8")
            GATB = sb.tile([128, 1024], bf16, tag="GATB")
            SSB = sb.tile([128, 256], f32, tag="SSB")
            SSBH = sb.tile([128, 256], bf16, tag="SSBH")
            SSB8 = sb.tile([128, 256], fp8, tag="SSB8")
            SE = sb.tile([128, 64], f32, tag="SE")
            RCP = sb.tile([128, 64], f32, tag="RCP")
            RCPB = sb.tile([128, 64], bf16, tag="RCPB")
            VBD = sb.tile([128, 2048], bf16, tag="VBD")
            S2 = sb.tile([128, 256], bf16, tag="S2")
            SSP1 = sb.tile([128, 256], f32, tag="SSP1")
            RCP1 = sb.tile([128, 256], f32, tag="RCP1")
            LNV = sb.tile([128, 256], f32, tag="LNV")
            RSQ = sb.tile([128, 256], f32, tag="RSQ")
            T1 = sb.tile([128, 256], f32, tag="T1")
            SCL = sb.tile([128, 256], f32, tag="SCL")
            VSB = sb.tile([128, 256], bf16, tag="VSB")
            VSF = sb.tile([128, 256], f32, tag="VSF")
            MAG = sb.tile([128, 1], mybir.dt.int32, tag="MAG")
            nc.vector.memset(MAG[:], 0x5F3759DF)

            def exchange(t, rep):
                """SSB (partial s) -> cross-core sum back into SSB."""
                use_fp8 = p["ag_fp8"] and t < 3 and exch == "ag"
                if exch == "ag":
                    if use_fp8:
                        nc.vector.tensor_copy(SSB8[:], SSB[:])
                        ag_in = dram.tile([128, 256], fp8, tag="ag_in",
                                          name=f"ag_in_{rep}_{t}")
                        ag_out = dram.tile([1024, 256], fp8, tag="ag_out",
                                           name=f"ag_out_{rep}_{t}")
                        nc.sync.dma_start(ag_in[:], SSB8[:])
                    else:
                        nc.vector.tensor_copy(SSBH[:], SSB[:])
                        ag_in = dram.tile([128, 256], bf16, tag="ag_in",
                                          name=f"ag_in_{rep}_{t}")
                        ag_out = dram.tile([1024, 256], bf16, tag="ag_out",
                                           name=f"ag_out_{rep}_{t}")
                        nc.sync.dma_start(ag_in[:], SSBH[:])
                    nc.gpsimd.collective_compute(
                        "AllGather", ALU.bypass,
                        replica_groups=[list(range(N_CORES))],
                        ins=[ag_in[:].opt()], outs=[ag_out[:].opt()],
                    )
                    if use_fp8:
                        nc.sync.dma_start(
                            GAT8[:].rearrange("p (r f) -> p r f", r=8),
                            ag_out[:].rearrange("(r p) f -> p r f", p=128),
                        )
                        nc.vector.tensor_add(GATB[:], GAT8[:, 0:1024],
                                             GAT8[:, 1024:2048])
                        nc.vector.tensor_add(GATB[:, 0:512], GATB[:, 0:512],
                                             GATB[:, 512:1024])
                        nc.vector.tensor_add(SSB[:], GATB[:, 0:256],
                                             GATB[:, 256:512])
                    else:
                        nc.sync.dma_start(
                            GAT[:].rearrange("p (r f) -> p r f", r=8),
                            ag_out[:].rearrange("(r p) f -> p r f", p=128),
                        )
                        g3 = GAT[:].rearrange("p (r f) -> p r f", r=8)
                        nc.vector.tensor_add(g3[:, 0:4], g3[:, 0:4], g3[:, 4:8])
                        nc.vector.tensor_add(g3[:, 0:2], g3[:, 0:2], g3[:, 2:4])
                        nc.vector.tensor_add(SSB[:], GAT[:, 0:256],
                                             GAT[:, 256:512])
                elif exch == "ar":
                    ar_in = dram.tile([128, 256], f32, tag="ar_in",
                                      name=f"ar_in_{rep}_{t}")
                    ar_out = dram.tile([128, 256], f32, tag="ar_out",
                                       name=f"ar_out_{rep}_{t}")
                    nc.gpsimd.dma_start(ar_in[:], SSB[:])
                    nc.gpsimd.collective_compute(
                        "AllReduce", ALU.add,
                        replica_groups=[list(range(N_CORES))],
                        ins=[ar_in[:].opt()], outs=[ar_out[:].opt()],
                    )
                    nc.gpsimd.dma_start(SSB[:], ar_out[:])
                elif exch == "noexch":
                    pass  # timing-only: skip the cross-core exchange

            def squash(t, rep):
                exchange(t, rep)
                # ss = sum_j s^2 (dup'd over 16 via DLT matmul)
                nc.vector.tensor_mul(S2[:], SSB[:], SSB[:])
                ps_sq = ps_pool_wv.tile([128, 256], f32, tag="wvp",
                                        name=f"ps_sq_{rep}_{t}")
                nc.tensor.matmul(ps_sq[:], DLT[:], S2[:], start=True, stop=True)
                if p["sq_pow"]:
                    # scale = (ss/(1+ss)) * rsqrt(ss+eps), all on DVE.
                    # rsqrt via quake seed + 2 Newton steps (no ACT tables).
                    i32 = mybir.dt.int32
                    nc.vector.tensor_scalar(
                        T1[:].bitcast(i32), ps_sq[:].bitcast(i32), 1, None,
                        op0=ALU.logical_shift_right)
                    nc.vector.tensor_tensor(
                        RSQ[:].bitcast(i32), MAG[:].broadcast_to([128, 256]),
                        T1[:].bitcast(i32), op=ALU.subtract)
                    nc.vector.tensor_scalar(LNV[:], ps_sq[:], -0.5,
                                            -0.5 * EPS, op0=ALU.mult,
                                            op1=ALU.add)
                    for _ in range(2):
                        nc.vector.tensor_mul(T1[:], RSQ[:], RSQ[:])
                        nc.vector.tensor_mul(T1[:], LNV[:], T1[:])
                        nc.vector.tensor_scalar_add(T1[:], T1[:], 1.5)
                        nc.vector.tensor_mul(RSQ[:], RSQ[:], T1[:])
                    nc.vector.tensor_scalar_add(SSP1[:], ps_sq[:], 1.0)
                    nc.vector.reciprocal(RCP1[:], SSP1[:])
                    nc.vector.tensor_mul(T1[:], ps_sq[:], RCP1[:])
                    nc.vector.tensor_mul(SCL[:], T1[:], RSQ[:])
                else:
                    # rsqrt via exp(-0.5*ln)
                    nc.vector.tensor_scalar_add(SSP1[:], ps_sq[:], 1.0)
                    nc.vector.reciprocal(RCP1[:], SSP1[:])
                    nc.vector.tensor_scalar_add(LNV[:], ps_sq[:], EPS)
                    nc.scalar.activation(LNV[:], LNV[:], AF.Ln)
                    nc.scalar.activation(RSQ[:], LNV[:], AF.Exp, scale=-0.5)
                    nc.vector.tensor_mul(T1[:], RCP1[:], RSQ[:])
                    nc.vector.tensor_mul(SCL[:], ps_sq[:], T1[:])
                if t < 3:
                    nc.vector.tensor_mul(VSB[:], SSB[:], SCL[:])
                    vsb_b = (VSB[:].rearrange("q (oc b) -> q oc b", oc=4)
                             .unsqueeze(2).broadcast_to([128, 4, 8, 64]))
                    mbd4 = MBD[:].rearrange("q (oc o8 b) -> q oc o8 b",
                                            oc=4, o8=8)
                    vbd4 = VBD[:].rearrange("q (oc o8 b) -> q oc o8 b",
                                            oc=4, o8=8)
                    nc.vector.tensor_mul(vbd4, vsb_b, mbd4)
                else:
                    nc.vector.tensor_mul(VSF[:], SSB[:], SCL[:])
                    nc.sync.dma_start(vout_d[:], VSF[:])

            def drain(eng_ch, dst, src):
                if wscale != 1.0:
                    if eng_ch == "A":
                        nc.scalar.mul(dst, src, 1.0 / wscale)
                    elif eng_ch == "D":
                        nc.vector.tensor_scalar_mul(dst, src, 1.0 / wscale)
                    else:
                        nc.gpsimd.tensor_scalar_mul(dst, src, 1.0 / wscale)
                else:
                    if eng_ch == "A":
                        nc.scalar.copy(dst, src)
                    elif eng_ch == "D":
                        nc.vector.tensor_copy(dst, src)
                    else:
                        nc.gpsimd.tensor_copy(dst, src)

            def eng_of(ch):
                return nc.vector if ch == "D" else nc.gpsimd

            for rep in range(reps):
                # ---------- iteration 1: s1 = (1/32) sum_i u_hat ----------
                ps_s1 = ps_pool_s.tile([128, 2048], f32, tag="ps_s",
                                       name=f"ps_s0_{rep}")
                for Gq in range(NG):
                    for k in range(8):
                        for oc in range(4):
                            nc.tensor.matmul(
                                ps_s1[:, oc * 512 : oc * 512 + 64],
                                WBs((Gq * 8 + k) * 4 + oc),
                                UY[:, (Gq * 8 + k) * 64 :
                                      (Gq * 8 + k) * 64 + 64],
                                start=(Gq == 0 and k == 0),
                                stop=(Gq == NG - 1 and k == 7),
                            )
                for oc in range(4):
                    nc.scalar.mul(SSB[:, oc * 64 : oc * 64 + 64],
                                  ps_s1[:, oc * 512 : oc * 512 + 64],
                                  1.0 / (32.0 * wscale))
                squash(1, rep)

                # ---------- iterations 2, 3 ----------
                for t in (2, 3):
                    ps_s = ps_pool_s.tile([128, 2048], f32, tag="ps_s",
                                          name=f"ps_s{rep}_{t}")

                    def wv_phase(Gq):
                        WVG = WVP[Gq % 2]
                        for k in range(8):
                            for ocp in range(2) if "skipwv" not in vset else []:
                                wvp = ps_pool_wv.tile(
                                    [128, 1024], f32, tag="wvp",
                                    name=f"wvp_{rep}_{t}_{Gq}_{k}_{ocp}")
                                for kk in range(2):
                                    oc = ocp * 2 + kk
                                    nc.tensor.matmul(
                                        wvp[:, kk * 512 : kk * 512 + 512],
                                        WD[:, ((oc * 4 + Gq) * 8 + k) * 128 :
                                              ((oc * 4 + Gq) * 8 + k) * 128
                                              + 128],
                                        VBD[:, oc * 512 : oc * 512 + 512],
                                        start=True, stop=True,
                                    )
                                dst = WVG[:, k * 2048 + ocp * 1024 :
                                             k * 2048 + ocp * 1024 + 1024]
                                drain(p["drain"][k * 2 + ocp], dst, wvp[:])

                    def post_a(Gq):
                        WVG = WVP[Gq % 2]
                        wvg4 = WVG[:].rearrange("p (k x b) -> p k x b",
                                                k=8, x=32)
                        uyg4 = (UY[:, Gq * 512 : Gq * 512 + 512]
                                .rearrange("p (k b) -> p k b", k=8)
                                .unsqueeze(2).broadcast_to([128, 8, 32, 64]))
                        # k-slab pair view for the strided halving tree:
                        # [p, kp, two, x] with two = adjacent k-slabs
                        wpair = WVG[:].rearrange(
                            "p (kp two x) -> p kp two x", two=2, x=2048)
                        if "skipmt" not in vset:
                            # a-path mul in two k-halves (chases the drains),
                            # paired k-sum tree right behind each half
                            nc.vector.tensor_mul(
                                wvg4[:, 0:4], wvg4[:, 0:4], uyg4[:, 0:4])
                            nc.vector.tensor_add(
                                wpair[:, 0:2, 0], wpair[:, 0:2, 0],
                                wpair[:, 0:2, 1])
                            nc.vector.tensor_mul(
                                wvg4[:, 4:8], wvg4[:, 4:8], uyg4[:, 4:8])
                            nc.vector.tensor_add(
                                wpair[:, 2:4, 0], wpair[:, 2:4, 0],
                                wpair[:, 2:4, 1])
                            # quads: (k01)+=(k23) at slabs 0,8192; +4096 src
                            wq = WVG[:].rearrange(
                                "p (kq rest) -> p kq rest", kq=2)
                            nc.vector.tensor_add(
                                wq[:, :, 0:2048], wq[:, :, 0:2048],
                                wq[:, :, 4096:6144])
                            # final: AG = k0123 + k4567
                            nc.vector.tensor_add(
                                AG[:], WVG[:, 0:2048], WVG[:, 8192:10240])
                        gsl = slice(Gq * 2048, Gq * 2048 + 2048)
                        if t == 2:
                            nc.vector.tensor_copy(BLOG[:, gsl], AG[:])
                        else:
                            nc.vector.tensor_add(AG[:], BLOG[:, gsl], AG[:])
                        nc.scalar.activation(EG[:], AG[:], AF.Exp)

                    def post_b(Gq):
                        WVG = WVP[Gq % 2]
                        wvg4 = WVG[:].rearrange("p (k x b) -> p k x b",
                                                k=8, x=32)
                        # sum over o: contiguous halving tree (o is outer)
                        nc.vector.tensor_add(SMT[:], EG[:, 0:1024],
                                             EG[:, 1024:2048])
                        nc.vector.tensor_add(SMT[:, 0:512], SMT[:, 0:512],
                                             SMT[:, 512:1024])
                        nc.vector.tensor_add(SMT[:, 0:256], SMT[:, 0:256],
                                             SMT[:, 256:512])
                        nc.vector.tensor_add(SMT[:, 0:128], SMT[:, 0:128],
                                             SMT[:, 128:256])
                        nc.vector.tensor_add(SE[:], SMT[:, 0:64],
                                             SMT[:, 64:128])
                        nc.vector.reciprocal(RCP[:], SE[:])
                        # urc = u * (1/Z): folds softmax denom into cu
                        # (RCP stays f32; avoids an ACT round-trip)
                        nc.vector.tensor_mul(
                            URC[:].rearrange("p (k b) -> p k b", k=8),
                            UY[:, Gq * 512 : Gq * 512 + 512]
                               .rearrange("p (k b) -> p k b", k=8),
                            RCP[:].unsqueeze(1).broadcast_to([128, 8, 64]))
                        # cu = e * urc (into WVG, now dead), built per-k so
                        # the s-path matmuls chase each chunk on PE
                        egb4 = (EG[:].rearrange("p (x b) -> p x b", x=32)
                                .unsqueeze(1).broadcast_to([128, 8, 32, 64]))
                        urc4 = (URC[:].rearrange("p (k b) -> p k b", k=8)
                                .unsqueeze(2).broadcast_to([128, 8, 32, 64]))
                        cks = set(p["cu_pool_ks"])
                        # Pool chunks issued first (slow engine, runs in
                        # parallel with DVE's chunks)
                        if "skipcu" not in vset:
                            for k in sorted(cks):
                                nc.gpsimd.tensor_mul(
                                    wvg4[:, k:k+1], egb4[:, k:k+1],
                                    urc4[:, k:k+1])
                        def s_mm(k):
                            for oc in range(4):
                                nc.tensor.matmul(
                                    ps_s[:, oc * 512 : oc * 512 + 512],
                                    WBs((Gq * 8 + k) * 4 + oc),
                                    WVG[:, k * 2048 + oc * 512 :
                                           k * 2048 + oc * 512 + 512],
                                    start=(Gq == 0 and k == 0),
                                    stop=(Gq == NG - 1 and k == 7),
                                )
                        dks = [k for k in range(8) if k not in cks]
                        for k in dks:
                            if "skipcu" not in vset:
                                nc.vector.tensor_mul(
                                    wvg4[:, k:k+1], egb4[:, k:k+1],
                                    urc4[:, k:k+1])
                            if "skipsmm" not in vset:
                                s_mm(k)
                        if "skipsmm" not in vset:
                            for k in sorted(cks):
                                s_mm(k)

                    # software pipeline: wv(G) is issued first so its PE
                    # matmuls and ACT/Pool drains are runnable immediately;
                    # post_a/post_b(G-1) then stream on DVE.
                    for Gq in range(NG + 1):
                        if Gq < NG:
                            wv_phase(Gq)
                        if Gq >= 1:
                            post_a(Gq - 1)
                            post_b(Gq - 1)
                    # fused diag extract via mask
                    if "skipsmm" not in vset:
                        for oc in range(4):
                            nc.vector.tensor_mul(
                                MSKD[:], ps_s[:, oc * 512 : oc * 512 + 512],
                                MSK[:])
                            nc.vector.tensor_reduce(
                                SSB[:, oc * 64 : oc * 64 + 64],
                                MSKD[:].rearrange("q (o8 b) -> q b o8", o8=8),
                                axis=mybir.AxisListType.X, op=ALU.add)
                    squash(t, rep)

    nc.compile()
    return nc


def _host_prep(u, W, plan=None):
    """Build per-core input maps (all host-side permutes)."""
    import concourse.mybir as mybir

    p = dict(PLAN)
    if plan:
        p.update(plan)
    wnp = mybir.dt.np(mybir.dt.float8e4) if p["wfp8"] else _BF16
    wscale = WS if p["wfp8"] else 1.0

    in_maps = []
    q = np.arange(128)
    dlt = (q[:, None] // 16 == q[None, :] // 16).astype(_BF16)
    o8p = np.arange(8)
    diag = (q[:, None] // 16 == o8p[None, :])
    mbd = np.ascontiguousarray(
        np.broadcast_to(diag[:, None, :, None], (128, 4, 8, 64))
    ).reshape(128, 2048).astype(_BF16)
    msk = np.ascontiguousarray(
        np.broadcast_to(diag[:, :, None], (128, 8, 64))
    ).reshape(128, 512).astype(np.float32) / wscale
    for c in range(N_CORES):
        Ws = np.asarray(W[c * I_LOC : (c + 1) * I_LOC], dtype=np.float32)
        Ws = Ws * wscale
        us = np.asarray(u[:, c * I_LOC : (c + 1) * I_LOC, :], dtype=np.float32)
        Wr = Ws.reshape(NG, 128, 4, 8, 16, 8)           # [Gq,p,oc,o8,j,k]
        wd = np.ascontiguousarray(
            Wr.transpose(3, 4, 2, 0, 5, 1)              # [o8,j,oc,Gq,k,p]
        ).reshape(128, 16384).astype(wnp)
        wb = np.ascontiguousarray(
            Wr.transpose(1, 0, 5, 2, 3, 4)              # [p,Gq,k,oc,o8,j]
        ).reshape(128, 16384).astype(wnp)
        ur = us.reshape(B, NG, 128, 8)                  # [b,Gq,p,k]
        uy = np.ascontiguousarray(
            ur.transpose(2, 1, 3, 0)                    # [p,Gq,k,b]
        ).reshape(128, 2048).astype(_BF16)
        in_maps.append({"wd": wd, "wb": wb, "uy": uy, "dlt": dlt,
                        "mbd": mbd, "msk": msk})
    return in_maps


def kernel(u, W):
    from concourse.bass_utils import run_bass_kernel_spmd

    if "nc" not in _CACHE:
        _CACHE["nc"] = _build_program(variant="ag")
    nc = _CACHE["nc"]

    in_maps = _host_prep(u, W)
    res = run_bass_kernel_spmd(
        nc, in_maps, core_ids=list(range(N_CORES)),
        trace=bool(int(os.environ.get("CAPS_TRACE", "0"))),
    )
    if isinstance(res, tuple):
        results = res[0]
    else:
        _CACHE["last_results"] = res
        results = res.results
    vout = results[0]["vout"]  # [128, 256]; identical on every core
    t = vout.reshape(8, 16, 4, 64)          # [o8, j, oc, b]
    v = np.ascontiguousarray(t.transpose(3, 2, 0, 1)).reshape(B, OUT_CAPS, OUT_DIM)
    return v.astype(np.float32)



# revision 8
# speedup vs baseline: 1.0375x; 1.0375x over previous
"""CapsNet routing-by-agreement kernel for 8 TRN2 NeuronCores.

Strategy (in_caps sharded 8-way):
  - Each core owns I_loc = 512 in_caps. Its W shard lives entirely in SBUF
    (two layouts), so routing iterations do ZERO HBM traffic for W / u_hat.
    u_hat is never materialized; each routing iteration recomputes the two
    W contractions on the TensorEngine with 128-deep packed contractions:
      a-path:  Wv[b,i,o,k] = sum_j W[i,o,j,k] v[b,o,j]   (contract (o8,j)=128,
               block-diag v as stationary operand)
               a[b,i,o]    = sum_k u[b,i,k] Wv[b,i,o,k]  (DVE mul + add-tree)
      s-path:  s[b,o,j]    = sum_{i,k} (c*u)[..] W[..]   (contract i mod 128,
               PSUM-accum over (i-block, k); 8x block-diag fanout over o8,
               diagonal extracted with a constant mask)
  - The only cross-core data is the per-out-capsule sum s [64,32,16]:
    AllGather (fp8 for routing iterations, bf16 for the final one) + local
    on-chip sum, once per routing iteration (3x).
  - Elementwise work is load-balanced across DVE / ACT / Pool(GPSIMD) via
    the PLAN knobs; PSUM drains rotate over all three engines.
  - All layout permutations of the inputs are done host-side in numpy; every
    on-chip tensor is DMA'd contiguously.

Index conventions (per core c): i_glob = c*512 + Gq*128 + p (Gq in 0..3,
p in 0..127);  o = oc*8 + o8 (oc in 0..3);  q = o8*16 + j (j in 0..15).

Host layouts:
  wd  [128,16384] : wd[q, ((oc*4+Gq)*8+k)*128 + p]  = W[i,o,j,k] * WS
  wb  [128,16384] : wb[p, ((Gq*8+k)*4+oc)*128 + q]  = W[i,o,j,k] * WS
  uy  [128, 2048] bf16 : uy[p, (Gq*8+k)*64 + b]     = u[b,i,k]
  dlt [128,  128] bf16 : dlt[q, m] = (q//16 == m//16)  (squash sum_j)
  mbd [128, 2048] bf16 : mbd[q, (oc,o8p,b)] = (q//16 == o8p)
  msk [128,  512] f32  : msk[q, (o8p,b)]   = (q//16 == o8p) / WS
Output:
  vout [128, 256] f32 : vout[o8*16+j, oc*64+b] = v[b, oc*8+o8, j]
"""

import json
import os
import sys

import numpy as np
import ml_dtypes

sys.path.insert(0, "/opt/trn_rl_repo")

B, IN_CAPS, IN_DIM = 64, 4096, 8
OUT_CAPS, OUT_DIM = 32, 16
EPS = 1e-8
N_CORES = 8
I_LOC = IN_CAPS // N_CORES  # 512
NG = I_LOC // 128           # 4

_BF16 = ml_dtypes.bfloat16

# Engine-assignment plan. Letters: A=ACT(scalar), D=DVE(vector), P=Pool(gpsimd)
PLAN = {
    # one char per wv-phase drain chunk, idx = k*2 + ocp (16 chunks of
    # [128,1024] f32 PSUM -> bf16 SBUF per Gq). Pool/GPSIMD cannot read
    # PSUM (BIR verifier), so only A/D are valid here.
    "drain": "AAAAAAAAAAAAAAAA",
    # which k's (0..7) of the cu build (c*u) go to Pool
    "cu_pool_ks": (5, 6, 7),
    # number of TAIL k's (0 or 2) of the a-path mul (Wv*u) + their pair
    # add that run on Pool instead of DVE
    "amul_pool": 0,
    # squash via fused tensor_scalar pow (vs Ln/Exp on ACT)
    "sq_pow": True,
    # newton refinement steps for the quake rsqrt seed (1 => ~0.17% max err)
    "sq_newton": 1,
    # fp8 payload for the routing AllGathers (iterations 1, 2).
    # Measured on HW: costs ~1.3e-2 of rel err — too close to the 2e-2
    # gate to enable by default.
    "ag_fp8": False,
    # W layouts in fp8e4 (scaled by WS), stationary-only. Measured on HW:
    # ~2.4e-2 rel err through routing feedback — keep off.
    "wfp8": False,
}
if os.environ.get("CAPS_PLAN"):
    PLAN.update(json.loads(os.environ["CAPS_PLAN"]))

WS = 16.0  # host-side W scale for fp8 dynamic range (descaled on chip)

_CACHE = {}


def _build_program(reps=1, variant="full", plan=None):
    import concourse.bass as bass
    import concourse.bacc as bacc
    import concourse.mybir as mybir
    import concourse.tile as tile
    from contextlib import ExitStack

    p = dict(PLAN)
    if plan:
        p.update(plan)

    f32 = mybir.dt.float32
    bf16 = mybir.dt.bfloat16
    fp8 = mybir.dt.float8e4
    wdt = fp8 if p["wfp8"] else bf16
    wscale = WS if p["wfp8"] else 1.0
    vset = set(variant.split(","))
    exch = ("ag" if "ag" in vset else
            "ar" if "ar" in vset else
            "noexch" if "noexch" in vset else "ag")
    AF = mybir.ActivationFunctionType
    ALU = mybir.AluOpType

    nc = bacc.Bacc(
        "TRN2",
        target_bir_lowering=False,
        debug=False,
        enable_asserts=False,
        num_devices=N_CORES,
    )

    wd_d = nc.dram_tensor("wd", [128, 16384], wdt, kind="ExternalInput")
    wb_d = nc.dram_tensor("wb", [128, 16384], wdt, kind="ExternalInput")
    uy_d = nc.dram_tensor("uy", [128, 2048], bf16, kind="ExternalInput")
    dlt_d = nc.dram_tensor("dlt", [128, 128], bf16, kind="ExternalInput")
    mbd_d = nc.dram_tensor("mbd", [128, 2048], bf16, kind="ExternalInput")
    msk_d = nc.dram_tensor("msk", [128, 512], f32, kind="ExternalInput")
    vout_d = nc.dram_tensor("vout", [128, 256], f32, kind="ExternalOutput")

    with tile.TileContext(nc) as tc:
        with ExitStack() as ctx:
            sb = ctx.enter_context(tc.tile_pool(name="sb", bufs=1))
            ps_pool_s = ctx.enter_context(
                tc.tile_pool(name="ps_s", bufs=1, space="PSUM"))
            ps_pool_wv = ctx.enter_context(
                tc.tile_pool(name="ps_wv", bufs=2, space="PSUM"))
            dram = ctx.enter_context(tc.tile_pool(name="dram", bufs=2,
                                                  space="DRAM"))

            WD = sb.tile([128, 16384], wdt, tag="WD")
            WBH = [sb.tile([128, 8192], wdt, tag=f"WB{i}", name=f"WB{i}")
                   for i in range(2)]
            UY = sb.tile([128, 2048], bf16, tag="UY")
            DLT = sb.tile([128, 128], bf16, tag="DLT")
            MBD = sb.tile([128, 2048], bf16, tag="MBD")
            MSK = sb.tile([128, 512], f32, tag="MSK")

            # DMA order: iteration-1 needs WB halves + UY first; WD (a-path)
            # is only needed after the first AllGather; constants in between.
            nc.sync.dma_start(WBH[0][:], wb_d[:, 0:8192])
            nc.sync.dma_start(UY[:, 0:1024], uy_d[:, 0:1024])
            nc.sync.dma_start(WBH[1][:], wb_d[:, 8192:16384])
            nc.sync.dma_start(UY[:, 1024:2048], uy_d[:, 1024:2048])
            nc.sync.dma_start(DLT[:], dlt_d[:])
            nc.sync.dma_start(MBD[:], mbd_d[:])
            nc.sync.dma_start(MSK[:], msk_d[:])
            nc.sync.dma_start(WD[:], wd_d[:])

            def WBs(idx):
                half, off = divmod(idx * 128, 8192)
                return WBH[half][:, off : off + 128]

            # big per-G scratch, parity double-buffered:
            # wv (drains) -> uwv (in-place mul) -> tree partials -> cu
            WVP = [sb.tile([128, 16384], bf16, tag=f"WV{i}", name=f"WV{i}")
                   for i in range(2)]
            BLOG = sb.tile([128, 8192], bf16, tag="BLOG")
            AG = sb.tile([128, 2048], bf16, tag="AG")
            # EG parity-double-buffered so exp(G) overlaps post_b(G-1) reads
            EGP = [sb.tile([128, 2048], bf16, tag=f"EG{i}", name=f"EG{i}")
                   for i in range(2)]
            URC = sb.tile([128, 512], bf16, tag="URC")
            SMT = sb.tile([128, 1024], bf16, tag="SMT")
            MSKD = sb.tile([128, 512], f32, tag="MSKD")
            GAT = sb.tile([128, 2048], bf16, tag="GAT")
            GAT8 = sb.tile([128, 2048], fp8, tag="GAT8")
            GATB = sb.tile([128, 1024], bf16, tag="GATB")
            SSB = sb.tile([128, 256], f32, tag="SSB")
            SSBH = sb.tile([128, 256], bf16, tag="SSBH")
            SSB8 = sb.tile([128, 256], fp8, tag="SSB8")
            SE = sb.tile([128, 64], f32, tag="SE")
            RCP = sb.tile([128, 64], f32, tag="RCP")
            RCPB = sb.tile([128, 64], bf16, tag="RCPB")
            VBD = sb.tile([128, 2048], bf16, tag="VBD")
            S2 = sb.tile([128, 256], bf16, tag="S2")
            SSP1 = sb.tile([128, 256], f32, tag="SSP1")
            RCP1 = sb.tile([128, 256], f32, tag="RCP1")
            LNV = sb.tile([128, 256], f32, tag="LNV")
            RSQ = sb.tile([128, 256], f32, tag="RSQ")
            T1 = sb.tile([128, 256], f32, tag="T1")
            SCL = sb.tile([128, 256], f32, tag="SCL")
            VSB = sb.tile([128, 256], bf16, tag="VSB")
            VSF = sb.tile([128, 256], f32, tag="VSF")
            MAG = sb.tile([128, 1], mybir.dt.int32, tag="MAG")
            nc.vector.memset(MAG[:], 0x5F3759DF)

            def exchange(t, rep):
                """SSB (partial s) -> cross-core sum back into SSB."""
                use_fp8 = p["ag_fp8"] and t < 3 and exch == "ag"
                if exch == "ag":
                    if use_fp8:
                        nc.vector.tensor_copy(SSB8[:], SSB[:])
                        ag_in = dram.tile([128, 256], fp8, tag="ag_in",
                                          name=f"ag_in_{rep}_{t}")
                        ag_out = dram.tile([1024, 256], fp8, tag="ag_out",
                                           name=f"ag_out_{rep}_{t}")
                        nc.sync.dma_start(ag_in[:], SSB8[:])
                    else:
                        nc.vector.tensor_copy(SSBH[:], SSB[:])
                        ag_in = dram.tile([128, 256], bf16, tag="ag_in",
                                          name=f"ag_in_{rep}_{t}")
                        ag_out = dram.tile([1024, 256], bf16, tag="ag_out",
                                           name=f"ag_out_{rep}_{t}")
                        nc.sync.dma_start(ag_in[:], SSBH[:])
                    nc.gpsimd.collective_compute(
                        "AllGather", ALU.bypass,
                        replica_groups=[list(range(N_CORES))],
                        ins=[ag_in[:].opt()], outs=[ag_out[:].opt()],
                    )
                    if use_fp8:
                        nc.sync.dma_start(
                            GAT8[:].rearrange("p (r f) -> p r f", r=8),
                            ag_out[:].rearrange("(r p) f -> p r f", p=128),
                        )
                        nc.vector.tensor_add(GATB[:], GAT8[:, 0:1024],
                                             GAT8[:, 1024:2048])
                        nc.vector.tensor_add(GATB[:, 0:512], GATB[:, 0:512],
                                             GATB[:, 512:1024])
                        nc.vector.tensor_add(SSB[:], GATB[:, 0:256],
                                             GATB[:, 256:512])
                    else:
                        nc.sync.dma_start(
                            GAT[:].rearrange("p (r f) -> p r f", r=8),
                            ag_out[:].rearrange("(r p) f -> p r f", p=128),
                        )
                        g3 = GAT[:].rearrange("p (r f) -> p r f", r=8)
                        nc.vector.tensor_add(g3[:, 0:4], g3[:, 0:4], g3[:, 4:8])
                        nc.vector.tensor_add(g3[:, 0:2], g3[:, 0:2], g3[:, 2:4])
                        nc.vector.tensor_add(SSB[:], GAT[:, 0:256],
                                             GAT[:, 256:512])
                elif exch == "ar":
                    ar_in = dram.tile([128, 256], f32, tag="ar_in",
                                      name=f"ar_in_{rep}_{t}")
                    ar_out = dram.tile([128, 256], f32, tag="ar_out",
                                       name=f"ar_out_{rep}_{t}")
                    nc.gpsimd.dma_start(ar_in[:], SSB[:])
                    nc.gpsimd.collective_compute(
                        "AllReduce", ALU.add,
                        replica_groups=[list(range(N_CORES))],
                        ins=[ar_in[:].opt()], outs=[ar_out[:].opt()],
                    )
                    nc.gpsimd.dma_start(SSB[:], ar_out[:])
                elif exch == "noexch":
                    pass  # timing-only: skip the cross-core exchange

            def squash(t, rep):
                exchange(t, rep)
                # ss = sum_j s^2 (dup'd over 16 via DLT matmul)
                nc.vector.tensor_mul(S2[:], SSB[:], SSB[:])
                ps_sq = ps_pool_wv.tile([128, 256], f32, tag="wvp",
                                        name=f"ps_sq_{rep}_{t}")
                nc.tensor.matmul(ps_sq[:], DLT[:], S2[:], start=True, stop=True)
                if p["sq_pow"]:
                    # scale = (ss/(1+ss)) * rsqrt(ss+eps), all on DVE.
                    # rsqrt via quake seed + 2 Newton steps (no ACT tables).
                    i32 = mybir.dt.int32
                    nc.vector.tensor_scalar(
                        T1[:].bitcast(i32), ps_sq[:].bitcast(i32), 1, None,
                        op0=ALU.logical_shift_right)
                    nc.vector.tensor_tensor(
                        RSQ[:].bitcast(i32), MAG[:].broadcast_to([128, 256]),
                        T1[:].bitcast(i32), op=ALU.subtract)
                    nc.vector.tensor_scalar(LNV[:], ps_sq[:], -0.5,
                                            -0.5 * EPS, op0=ALU.mult,
                                            op1=ALU.add)
                    for _ in range(p["sq_newton"]):
                        nc.vector.tensor_mul(T1[:], RSQ[:], RSQ[:])
                        nc.vector.tensor_mul(T1[:], LNV[:], T1[:])
                        nc.vector.tensor_scalar_add(T1[:], T1[:], 1.5)
                        nc.vector.tensor_mul(RSQ[:], RSQ[:], T1[:])
                    nc.vector.tensor_scalar_add(SSP1[:], ps_sq[:], 1.0)
                    nc.vector.reciprocal(RCP1[:], SSP1[:])
                    nc.vector.tensor_mul(T1[:], ps_sq[:], RCP1[:])
                    nc.vector.tensor_mul(SCL[:], T1[:], RSQ[:])
                else:
                    # rsqrt via exp(-0.5*ln)
                    nc.vector.tensor_scalar_add(SSP1[:], ps_sq[:], 1.0)
                    nc.vector.reciprocal(RCP1[:], SSP1[:])
                    nc.vector.tensor_scalar_add(LNV[:], ps_sq[:], EPS)
                    nc.scalar.activation(LNV[:], LNV[:], AF.Ln)
                    nc.scalar.activation(RSQ[:], LNV[:], AF.Exp, scale=-0.5)
                    nc.vector.tensor_mul(T1[:], RCP1[:], RSQ[:])
                    nc.vector.tensor_mul(SCL[:], ps_sq[:], T1[:])
                if t < 3:
                    nc.vector.tensor_mul(VSB[:], SSB[:], SCL[:])
                    vsb_b = (VSB[:].rearrange("q (oc b) -> q oc b", oc=4)
                             .unsqueeze(2).broadcast_to([128, 4, 8, 64]))
                    mbd4 = MBD[:].rearrange("q (oc o8 b) -> q oc o8 b",
                                            oc=4, o8=8)
                    vbd4 = VBD[:].rearrange("q (oc o8 b) -> q oc o8 b",
                                            oc=4, o8=8)
                    nc.vector.tensor_mul(vbd4, vsb_b, mbd4)
                else:
                    nc.vector.tensor_mul(VSF[:], SSB[:], SCL[:])
                    nc.sync.dma_start(vout_d[:], VSF[:])

            def drain(eng_ch, dst, src):
                if wscale != 1.0:
                    if eng_ch == "A":
                        nc.scalar.mul(dst, src, 1.0 / wscale)
                    elif eng_ch == "D":
                        nc.vector.tensor_scalar_mul(dst, src, 1.0 / wscale)
                    else:
                        nc.gpsimd.tensor_scalar_mul(dst, src, 1.0 / wscale)
                else:
                    if eng_ch == "A":
                        nc.scalar.copy(dst, src)
                    elif eng_ch == "D":
                        nc.vector.tensor_copy(dst, src)
                    else:
                        nc.gpsimd.tensor_copy(dst, src)

            def eng_of(ch):
                return nc.vector if ch == "D" else nc.gpsimd

            for rep in range(reps):
                # ---------- iteration 1: s1 = (1/32) sum_i u_hat ----------
                ps_s1 = ps_pool_s.tile([128, 2048], f32, tag="ps_s",
                                       name=f"ps_s0_{rep}")
                for Gq in range(NG):
                    for k in range(8):
                        for oc in range(4):
                            nc.tensor.matmul(
                                ps_s1[:, oc * 512 : oc * 512 + 64],
                                WBs((Gq * 8 + k) * 4 + oc),
                                UY[:, (Gq * 8 + k) * 64 :
                                      (Gq * 8 + k) * 64 + 64],
                                start=(Gq == 0 and k == 0),
                                stop=(Gq == NG - 1 and k == 7),
                            )
                for oc in range(4):
                    nc.scalar.mul(SSB[:, oc * 64 : oc * 64 + 64],
                                  ps_s1[:, oc * 512 : oc * 512 + 64],
                                  1.0 / (32.0 * wscale))
                squash(1, rep)

                # ---------- iterations 2, 3 ----------
                for t in (2, 3):
                    ps_s = ps_pool_s.tile([128, 2048], f32, tag="ps_s",
                                          name=f"ps_s{rep}_{t}")

                    def wv_half(Gq, ks):
                        WVG = WVP[Gq % 2]
                        for k in ks:
                            for ocp in range(2) if "skipwv" not in vset else []:
                                wvp = ps_pool_wv.tile(
                                    [128, 1024], f32, tag="wvp",
                                    name=f"wvp_{rep}_{t}_{Gq}_{k}_{ocp}")
                                for kk in range(2):
                                    oc = ocp * 2 + kk
                                    nc.tensor.matmul(
                                        wvp[:, kk * 512 : kk * 512 + 512],
                                        WD[:, ((oc * 4 + Gq) * 8 + k) * 128 :
                                              ((oc * 4 + Gq) * 8 + k) * 128
                                              + 128],
                                        VBD[:, oc * 512 : oc * 512 + 512],
                                        start=True, stop=True,
                                    )
                                dst = WVG[:, k * 2048 + ocp * 1024 :
                                             k * 2048 + ocp * 1024 + 1024]
                                drain(p["drain"][k * 2 + ocp], dst, wvp[:])

                    def post_a(Gq):
                        WVG = WVP[Gq % 2]
                        EG = EGP[Gq % 2]
                        wvg4 = WVG[:].rearrange("p (k x b) -> p k x b",
                                                k=8, x=32)
                        uyg4 = (UY[:, Gq * 512 : Gq * 512 + 512]
                                .rearrange("p (k b) -> p k b", k=8)
                                .unsqueeze(2).broadcast_to([128, 8, 32, 64]))
                        # k-slab pair view for the strided halving tree:
                        # [p, kp, two, x] with two = adjacent k-slabs
                        wpair = WVG[:].rearrange(
                            "p (kp two x) -> p kp two x", two=2, x=2048)
                        npk = p["amul_pool"]  # 0 or 2 tail ks on Pool
                        dk = 8 - npk
                        if "skipmt" not in vset:
                            # Pool takes the tail ks (slow engine, issued
                            # first so it overlaps the DVE halves)
                            if npk:
                                nc.gpsimd.tensor_mul(
                                    wvg4[:, dk:8], wvg4[:, dk:8],
                                    uyg4[:, dk:8])
                                nc.gpsimd.tensor_add(
                                    wpair[:, dk // 2 : 4, 0],
                                    wpair[:, dk // 2 : 4, 0],
                                    wpair[:, dk // 2 : 4, 1])
                            # a-path mul in two k-halves (chases the drains),
                            # paired k-sum tree right behind each half
                            nc.vector.tensor_mul(
                                wvg4[:, 0:4], wvg4[:, 0:4], uyg4[:, 0:4])
                            nc.vector.tensor_add(
                                wpair[:, 0:2, 0], wpair[:, 0:2, 0],
                                wpair[:, 0:2, 1])
                            if dk > 4:
                                nc.vector.tensor_mul(
                                    wvg4[:, 4:dk], wvg4[:, 4:dk],
                                    uyg4[:, 4:dk])
                                nc.vector.tensor_add(
                                    wpair[:, 2 : dk // 2, 0],
                                    wpair[:, 2 : dk // 2, 0],
                                    wpair[:, 2 : dk // 2, 1])
                            # quads: (k01)+=(k23) at slabs 0,8192; +4096 src
                            wq = WVG[:].rearrange(
                                "p (kq rest) -> p kq rest", kq=2)
                            nc.vector.tensor_add(
                                wq[:, :, 0:2048], wq[:, :, 0:2048],
                                wq[:, :, 4096:6144])
                        gsl = slice(Gq * 2048, Gq * 2048 + 2048)
                        if t == 2:
                            # final tree add lands directly in BLOG
                            nc.vector.tensor_add(
                                BLOG[:, gsl], WVG[:, 0:2048],
                                WVG[:, 8192:10240])
                            nc.scalar.activation(EG[:], BLOG[:, gsl], AF.Exp)
                        else:
                            nc.vector.tensor_add(
                                AG[:], WVG[:, 0:2048], WVG[:, 8192:10240])
                            nc.vector.tensor_add(AG[:], BLOG[:, gsl], AG[:])
                            nc.scalar.activation(EG[:], AG[:], AF.Exp)

                    def post_b(Gq):
                        WVG = WVP[Gq % 2]
                        EG = EGP[Gq % 2]
                        wvg4 = WVG[:].rearrange("p (k x b) -> p k x b",
                                                k=8, x=32)
                        # sum over o: contiguous halving tree (o is outer)
                        nc.vector.tensor_add(SMT[:], EG[:, 0:1024],
                                             EG[:, 1024:2048])
                        nc.vector.tensor_add(SMT[:, 0:512], SMT[:, 0:512],
                                             SMT[:, 512:1024])
                        nc.vector.tensor_add(SMT[:, 0:256], SMT[:, 0:256],
                                             SMT[:, 256:512])
                        nc.vector.tensor_add(SMT[:, 0:128], SMT[:, 0:128],
                                             SMT[:, 128:256])
                        nc.vector.tensor_add(SE[:], SMT[:, 0:64],
                                             SMT[:, 64:128])
                        nc.vector.reciprocal(RCP[:], SE[:])
                        # urc = u * (1/Z): folds softmax denom into cu
                        # (RCP stays f32; avoids an ACT round-trip)
                        nc.vector.tensor_mul(
                            URC[:].rearrange("p (k b) -> p k b", k=8),
                            UY[:, Gq * 512 : Gq * 512 + 512]
                               .rearrange("p (k b) -> p k b", k=8),
                            RCP[:].unsqueeze(1).broadcast_to([128, 8, 64]))
                        # cu = e * urc (into WVG, now dead), built per-k so
                        # the s-path matmuls chase each chunk on PE
                        egb4 = (EG[:].rearrange("p (x b) -> p x b", x=32)
                                .unsqueeze(1).broadcast_to([128, 8, 32, 64]))
                        urc4 = (URC[:].rearrange("p (k b) -> p k b", k=8)
                                .unsqueeze(2).broadcast_to([128, 8, 32, 64]))
                        cks = set(p["cu_pool_ks"])
                        # Pool chunks issued first (slow engine, runs in
                        # parallel with DVE's chunks)
                        if "skipcu" not in vset:
                            for k in sorted(cks):
                                nc.gpsimd.tensor_mul(
                                    wvg4[:, k:k+1], egb4[:, k:k+1],
                                    urc4[:, k:k+1])
                        def s_mm(k):
                            for oc in range(4):
                                nc.tensor.matmul(
                                    ps_s[:, oc * 512 : oc * 512 + 512],
                                    WBs((Gq * 8 + k) * 4 + oc),
                                    WVG[:, k * 2048 + oc * 512 :
                                           k * 2048 + oc * 512 + 512],
                                    start=(Gq == 0 and k == 0),
                                    stop=(Gq == NG - 1 and k == 7),
                                )
                        dks = [k for k in range(8) if k not in cks]
                        for k in dks:
                            if "skipcu" not in vset:
                                nc.vector.tensor_mul(
                                    wvg4[:, k:k+1], egb4[:, k:k+1],
                                    urc4[:, k:k+1])
                            if "skipsmm" not in vset:
                                s_mm(k)
                        if "skipsmm" not in vset:
                            for k in sorted(cks):
                                s_mm(k)

                    # software pipeline: wv(G) is split in halves with
                    # post_a(G-1) issued between them so exp(G-1) sits
                    # behind only 8 drains in the ACT FIFO, not 16.
                    for Gq in range(NG + 1):
                        if Gq < NG:
                            wv_half(Gq, range(0, 4))
                        if Gq >= 1:
                            post_a(Gq - 1)
                        if Gq < NG:
                            wv_half(Gq, range(4, 8))
                        if Gq >= 1:
                            post_b(Gq - 1)
                    # fused diag extract via mask
                    if "skipsmm" not in vset:
                        for oc in range(4):
                            nc.vector.tensor_mul(
                                MSKD[:], ps_s[:, oc * 512 : oc * 512 + 512],
                                MSK[:])
                            nc.vector.tensor_reduce(
                                SSB[:, oc * 64 : oc * 64 + 64],
                                MSKD[:].rearrange("q (o8 b) -> q b o8", o8=8),
                                axis=mybir.AxisListType.X, op=ALU.add)
                    squash(t, rep)

    nc.compile()
    return nc


def _host_prep(u, W, plan=None):
    """Build per-core input maps (all host-side permutes)."""
    import concourse.mybir as mybir

    p = dict(PLAN)
    if plan:
        p.update(plan)
    wnp = mybir.dt.np(mybir.dt.float8e4) if p["wfp8"] else _BF16
    wscale = WS if p["wfp8"] else 1.0

    in_maps = []
    q = np.arange(128)
    dlt = (q[:, None] // 16 == q[None, :] // 16).astype(_BF16)
    o8p = np.arange(8)
    diag = (q[:, None] // 16 == o8p[None, :])
    mbd = np.ascontiguousarray(
        np.broadcast_to(diag[:, None, :, None], (128, 4, 8, 64))
    ).reshape(128, 2048).astype(_BF16)
    msk = np.ascontiguousarray(
        np.broadcast_to(diag[:, :, None], (128, 8, 64))
    ).reshape(128, 512).astype(np.float32) / wscale
    for c in range(N_CORES):
        Ws = np.asarray(W[c * I_LOC : (c + 1) * I_LOC], dtype=np.float32)
        Ws = Ws * wscale
        us = np.asarray(u[:, c * I_LOC : (c + 1) * I_LOC, :], dtype=np.float32)
        Wr = Ws.reshape(NG, 128, 4, 8, 16, 8)           # [Gq,p,oc,o8,j,k]
        wd = np.ascontiguousarray(
            Wr.transpose(3, 4, 2, 0, 5, 1)              # [o8,j,oc,Gq,k,p]
        ).reshape(128, 16384).astype(wnp)
        wb = np.ascontiguousarray(
            Wr.transpose(1, 0, 5, 2, 3, 4)              # [p,Gq,k,oc,o8,j]
        ).reshape(128, 16384).astype(wnp)
        ur = us.reshape(B, NG, 128, 8)                  # [b,Gq,p,k]
        uy = np.ascontiguousarray(
            ur.transpose(2, 1, 3, 0)                    # [p,Gq,k,b]
        ).reshape(128, 2048).astype(_BF16)
        in_maps.append({"wd": wd, "wb": wb, "uy": uy, "dlt": dlt,
                        "mbd": mbd, "msk": msk})
    return in_maps


def kernel(u, W):
    from concourse.bass_utils import run_bass_kernel_spmd

    if "nc" not in _CACHE:
        _CACHE["nc"] = _build_program(variant="ag")
    nc = _CACHE["nc"]

    in_maps = _host_prep(u, W)
    res = run_bass_kernel_spmd(
        nc, in_maps, core_ids=list(range(N_CORES)),
        trace=bool(int(os.environ.get("CAPS_TRACE", "0"))),
    )
    if isinstance(res, tuple):
        results = res[0]
    else:
        _CACHE["last_results"] = res
        results = res.results
    vout = results[0]["vout"]  # [128, 256]; identical on every core
    t = vout.reshape(8, 16, 4, 64)          # [o8, j, oc, b]
    v = np.ascontiguousarray(t.transpose(3, 2, 0, 1)).reshape(B, OUT_CAPS, OUT_DIM)
    return v.astype(np.float32)



# revision 19
# speedup vs baseline: 1.0552x; 1.0171x over previous
"""CapsNet routing-by-agreement kernel for 8 TRN2 NeuronCores.

Strategy (in_caps sharded 8-way):
  - Each core owns I_loc = 512 in_caps. Its W shard lives entirely in SBUF
    (two layouts), so routing iterations do ZERO HBM traffic for W / u_hat.
    u_hat is never materialized; each routing iteration recomputes the two
    W contractions on the TensorEngine with 128-deep packed contractions:
      a-path:  Wv[b,i,o,k] = sum_j W[i,o,j,k] v[b,o,j]   (contract (o8,j)=128,
               block-diag v as stationary operand)
               a[b,i,o]    = sum_k u[b,i,k] Wv[b,i,o,k]  (DVE mul + add-tree)
      s-path:  s[b,o,j]    = sum_{i,k} (c*u)[..] W[..]   (contract i mod 128,
               PSUM-accum over (i-block, k); 8x block-diag fanout over o8,
               diagonal extracted with a constant mask)
  - The only cross-core data is the per-out-capsule sum s [64,32,16]:
    AllGather (fp8 for routing iterations, bf16 for the final one) + local
    on-chip sum, once per routing iteration (3x).
  - Elementwise work is load-balanced across DVE / ACT / Pool(GPSIMD) via
    the PLAN knobs; PSUM drains rotate over all three engines.
  - All layout permutations of the inputs are done host-side in numpy; every
    on-chip tensor is DMA'd contiguously.

Index conventions (per core c): i_glob = c*512 + Gq*128 + p (Gq in 0..3,
p in 0..127);  o = oc*8 + o8 (oc in 0..3);  q = o8*16 + j (j in 0..15).

Host layouts:
  wd  [128,16384] : wd[q, ((oc*4+Gq)*8+k)*128 + p]  = W[i,o,j,k] * WS
  wb  [128,16384] : wb[p, ((Gq*8+k)*4+oc)*128 + q]  = W[i,o,j,k] * WS
  uy  [128, 2048] bf16 : uy[p, (Gq*8+k)*64 + b]     = u[b,i,k]
  dlt [128,  128] bf16 : dlt[q, m] = (q//16 == m//16)  (squash sum_j)
  mbd [128, 2048] bf16 : mbd[q, (oc,o8p,b)] = (q//16 == o8p)
  msk [128,  512] f32  : msk[q, (o8p,b)]   = (q//16 == o8p) / WS
Output:
  vout [128, 256] f32 : vout[o8*16+j, oc*64+b] = v[b, oc*8+o8, j]
"""

import json
import os
import sys

import numpy as np
import ml_dtypes

sys.path.insert(0, "/opt/trn_rl_repo")

B, IN_CAPS, IN_DIM = 64, 4096, 8
OUT_CAPS, OUT_DIM = 32, 16
EPS = 1e-8
N_CORES = 8
I_LOC = IN_CAPS // N_CORES  # 512
NG = I_LOC // 128           # 4

_BF16 = ml_dtypes.bfloat16

# Engine-assignment plan. Letters: A=ACT(scalar), D=DVE(vector), P=Pool(gpsimd)
PLAN = {
    # one char per wv-phase drain chunk, idx = k*2 + ocp (16 chunks of
    # [128,1024] f32 PSUM -> bf16 SBUF per Gq). Pool/GPSIMD cannot read
    # PSUM (BIR verifier), so only A/D are valid here.
    "drain": "AAAAAAAAAAAAAAAA",
    # drain schedule for the FIRST Gq of each iteration (measured: DVE
    # shares regress — ACT-only wins)
    "drain_g0": "AAAAAAAAAAAAAAAA",
    # which k's (0..7) of the cu build (c*u) go to Pool
    "cu_pool_ks": (5, 6, 7),
    # same, for the LAST Gq: its cu chunks gate s_mm -> exchange, so Pool
    # lag there extends the iteration tail
    "cu_pool_ks_last": (),
    # number of TAIL k's (0 or 2) of the a-path mul (Wv*u) + their pair
    # add that run on Pool instead of DVE
    "amul_pool": 0,
    # squash via fused tensor_scalar pow (vs Ln/Exp on ACT)
    "sq_pow": True,
    # newton refinement steps for the quake rsqrt seed (1 => ~0.17% max err)
    "sq_newton": 1,
    # fp8 payload for the routing AllGathers (iterations 1, 2).
    # Measured on HW: costs ~1.3e-2 of rel err — too close to the 2e-2
    # gate to enable by default.
    "ag_fp8": False,
    # W layouts in fp8e4 (scaled by WS), stationary-only. Measured on HW:
    # ~2.4e-2 rel err through routing feedback — keep off.
    "wfp8": False,
}
if os.environ.get("CAPS_PLAN"):
    PLAN.update(json.loads(os.environ["CAPS_PLAN"]))

WS = 16.0  # host-side W scale for fp8 dynamic range (descaled on chip)

_CACHE = {}


def _build_program(reps=1, variant="full", plan=None):
    import concourse.bass as bass
    import concourse.bacc as bacc
    import concourse.mybir as mybir
    import concourse.tile as tile
    from contextlib import ExitStack

    p = dict(PLAN)
    if plan:
        p.update(plan)

    f32 = mybir.dt.float32
    bf16 = mybir.dt.bfloat16
    fp8 = mybir.dt.float8e4
    wdt = fp8 if p["wfp8"] else bf16
    wscale = WS if p["wfp8"] else 1.0
    vset = set(variant.split(","))
    exch = ("ag" if "ag" in vset else
            "ar" if "ar" in vset else
            "fakeexch" if "fakeexch" in vset else
            "noexch" if "noexch" in vset else "ag")
    AF = mybir.ActivationFunctionType
    ALU = mybir.AluOpType

    nc = bacc.Bacc(
        "TRN2",
        target_bir_lowering=False,
        debug=False,
        enable_asserts=False,
        num_devices=N_CORES,
    )

    wd_d = nc.dram_tensor("wd", [128, 16384], wdt, kind="ExternalInput")
    wb_d = nc.dram_tensor("wb", [128, 16384], wdt, kind="ExternalInput")
    uy_d = nc.dram_tensor("uy", [128, 2048], bf16, kind="ExternalInput")
    dlt_d = nc.dram_tensor("dlt", [128, 128], bf16, kind="ExternalInput")
    mbd_d = nc.dram_tensor("mbd", [128, 2048], bf16, kind="ExternalInput")
    msk_d = nc.dram_tensor("msk", [128, 512], f32, kind="ExternalInput")
    vout_d = nc.dram_tensor("vout", [128, 256], f32, kind="ExternalOutput")

    with tile.TileContext(nc) as tc:
        with ExitStack() as ctx:
            sb = ctx.enter_context(tc.tile_pool(name="sb", bufs=1))
            ps_pool_s = ctx.enter_context(
                tc.tile_pool(name="ps_s", bufs=1, space="PSUM"))
            ps_pool_wv = ctx.enter_context(
                tc.tile_pool(name="ps_wv", bufs=2, space="PSUM"))
            dram = ctx.enter_context(tc.tile_pool(name="dram", bufs=2,
                                                  space="DRAM"))

            WD = sb.tile([128, 16384], wdt, tag="WD")
            WBH = [sb.tile([128, 8192], wdt, tag=f"WB{i}", name=f"WB{i}")
                   for i in range(2)]
            UY = sb.tile([128, 2048], bf16, tag="UY")
            DLT = sb.tile([128, 128], bf16, tag="DLT")
            MBD = sb.tile([128, 2048], bf16, tag="MBD")
            MSK = sb.tile([128, 512], f32, tag="MSK")

            # DMA order: iteration-1 needs WB halves + UY first; WD (a-path)
            # is only needed after the first AllGather; constants in between.
            nc.sync.dma_start(WBH[0][:], wb_d[:, 0:8192])
            nc.sync.dma_start(UY[:, 0:1024], uy_d[:, 0:1024])
            nc.sync.dma_start(WBH[1][:], wb_d[:, 8192:16384])
            nc.sync.dma_start(UY[:, 1024:2048], uy_d[:, 1024:2048])
            nc.sync.dma_start(DLT[:], dlt_d[:])
            nc.sync.dma_start(MBD[:], mbd_d[:])
            nc.sync.dma_start(MSK[:], msk_d[:])
            nc.sync.dma_start(WD[:], wd_d[:])

            def WBs(idx):
                half, off = divmod(idx * 128, 8192)
                return WBH[half][:, off : off + 128]

            # big per-G scratch, parity double-buffered:
            # wv (drains) -> uwv (in-place mul) -> tree partials -> cu
            WVP = [sb.tile([128, 16384], bf16, tag=f"WV{i}", name=f"WV{i}")
                   for i in range(2)]
            BLOG = sb.tile([128, 8192], bf16, tag="BLOG")
            AG = sb.tile([128, 2048], bf16, tag="AG")
            # EG parity-double-buffered so exp(G) overlaps post_b(G-1) reads
            EGP = [sb.tile([128, 2048], bf16, tag=f"EG{i}", name=f"EG{i}")
                   for i in range(2)]
            # URC parity-buffered: Pool's cu(G-1) reads must not block the
            # DVE write of URC(G)
            URCP = [sb.tile([128, 512], bf16, tag=f"URC{i}", name=f"URC{i}")
                    for i in range(2)]
            SMT = sb.tile([128, 1024], bf16, tag="SMT")
            MSKD = sb.tile([128, 512], f32, tag="MSKD")
            GAT = sb.tile([128, 2048], bf16, tag="GAT")
            GAT8 = sb.tile([128, 2048], fp8, tag="GAT8")
            GATB = sb.tile([128, 1024], bf16, tag="GATB")
            SSB = sb.tile([128, 256], f32, tag="SSB")
            SSBH = sb.tile([128, 256], bf16, tag="SSBH")
            SSB8 = sb.tile([128, 256], fp8, tag="SSB8")
            SE = sb.tile([128, 64], f32, tag="SE")
            RCP = sb.tile([128, 64], f32, tag="RCP")
            RCPB = sb.tile([128, 64], bf16, tag="RCPB")
            VBD = sb.tile([128, 2048], bf16, tag="VBD")
            S2 = sb.tile([128, 256], bf16, tag="S2")
            SSP1 = sb.tile([128, 256], f32, tag="SSP1")
            RCP1 = sb.tile([128, 256], f32, tag="RCP1")
            LNV = sb.tile([128, 256], f32, tag="LNV")
            RSQ = sb.tile([128, 256], f32, tag="RSQ")
            T1 = sb.tile([128, 256], f32, tag="T1")
            SCL = sb.tile([128, 256], f32, tag="SCL")
            VSB = sb.tile([128, 256], bf16, tag="VSB")
            VSF = sb.tile([128, 256], f32, tag="VSF")
            MAG = sb.tile([128, 1], mybir.dt.int32, tag="MAG")
            nc.vector.memset(MAG[:], 0x5F3759DF)

            def exchange(t, rep):
                """SSB (partial s) -> cross-core sum back into SSB."""
                use_fp8 = p["ag_fp8"] and t < 3 and exch == "ag"
                if exch == "fakeexch":
                    # timing probe: same DMAs + adds as "ag", but the
                    # collective is replaced by a local 8x DRAM broadcast
                    nc.vector.tensor_copy(SSBH[:], SSB[:])
                    ag_in = dram.tile([128, 256], bf16, tag="ag_in",
                                      name=f"ag_in_{rep}_{t}")
                    ag_out = dram.tile([1024, 256], bf16, tag="ag_out",
                                       name=f"ag_out_{rep}_{t}")
                    nc.sync.dma_start(ag_in[:], SSBH[:])
                    nc.sync.dma_start(
                        ag_out[:].rearrange("(r p) f -> r p f", r=8),
                        ag_in[:].unsqueeze(0).broadcast_to([8, 128, 256]),
                    )
                    nc.sync.dma_start(
                        GAT[:].rearrange("p (r f) -> p r f", r=8),
                        ag_out[:].rearrange("(r p) f -> p r f", p=128),
                    )
                    g3 = GAT[:].rearrange("p (r f) -> p r f", r=8)
                    nc.vector.tensor_add(g3[:, 0:4], g3[:, 0:4], g3[:, 4:8])
                    nc.vector.tensor_add(g3[:, 0:2], g3[:, 0:2], g3[:, 2:4])
                    nc.vector.tensor_add(SSB[:], GAT[:, 0:256],
                                         GAT[:, 256:512])
                    return
                if exch == "ag":
                    if use_fp8:
                        nc.vector.tensor_copy(SSB8[:], SSB[:])
                        ag_in = dram.tile([128, 256], fp8, tag="ag_in",
                                          name=f"ag_in_{rep}_{t}")
                        ag_out = dram.tile([1024, 256], fp8, tag="ag_out",
                                           name=f"ag_out_{rep}_{t}")
                        nc.sync.dma_start(ag_in[:], SSB8[:])
                    else:
                        nc.vector.tensor_copy(SSBH[:], SSB[:])
                        ag_in = dram.tile([128, 256], bf16, tag="ag_in",
                                          name=f"ag_in_{rep}_{t}")
                        ag_out = dram.tile([1024, 256], bf16, tag="ag_out",
                                           name=f"ag_out_{rep}_{t}")
                        nc.sync.dma_start(ag_in[:], SSBH[:])
                    nc.gpsimd.collective_compute(
                        "AllGather", ALU.bypass,
                        replica_groups=[list(range(N_CORES))],
                        ins=[ag_in[:].opt()], outs=[ag_out[:].opt()],
                    )
                    if use_fp8:
                        nc.sync.dma_start(
                            GAT8[:].rearrange("p (r f) -> p r f", r=8),
                            ag_out[:].rearrange("(r p) f -> p r f", p=128),
                        )
                        nc.vector.tensor_add(GATB[:], GAT8[:, 0:1024],
                                             GAT8[:, 1024:2048])
                        nc.vector.tensor_add(GATB[:, 0:512], GATB[:, 0:512],
                                             GATB[:, 512:1024])
                        nc.vector.tensor_add(SSB[:], GATB[:, 0:256],
                                             GATB[:, 256:512])
                    else:
                        nc.sync.dma_start(
                            GAT[:].rearrange("p (r f) -> p r f", r=8),
                            ag_out[:].rearrange("(r p) f -> p r f", p=128),
                        )
                        g3 = GAT[:].rearrange("p (r f) -> p r f", r=8)
                        nc.vector.tensor_add(g3[:, 0:4], g3[:, 0:4], g3[:, 4:8])
                        nc.vector.tensor_add(g3[:, 0:2], g3[:, 0:2], g3[:, 2:4])
                        nc.vector.tensor_add(SSB[:], GAT[:, 0:256],
                                             GAT[:, 256:512])
                elif exch == "ar":
                    ar_in = dram.tile([128, 256], f32, tag="ar_in",
                                      name=f"ar_in_{rep}_{t}")
                    ar_out = dram.tile([128, 256], f32, tag="ar_out",
                                       name=f"ar_out_{rep}_{t}")
                    nc.gpsimd.dma_start(ar_in[:], SSB[:])
                    nc.gpsimd.collective_compute(
                        "AllReduce", ALU.add,
                        replica_groups=[list(range(N_CORES))],
                        ins=[ar_in[:].opt()], outs=[ar_out[:].opt()],
                    )
                    nc.gpsimd.dma_start(SSB[:], ar_out[:])
                elif exch == "noexch":
                    pass  # timing-only: skip the cross-core exchange

            def squash(t, rep):
                exchange(t, rep)
                # ss = sum_j s^2 (dup'd over 16 via DLT matmul)
                nc.vector.tensor_mul(S2[:], SSB[:], SSB[:])
                ps_sq = ps_pool_wv.tile([128, 256], f32, tag="wvp",
                                        name=f"ps_sq_{rep}_{t}")
                nc.tensor.matmul(ps_sq[:], DLT[:], S2[:], start=True, stop=True)
                if p["sq_pow"]:
                    # scale = (ss/(1+ss)) * rsqrt(ss+eps), all on DVE.
                    # rsqrt via quake seed + 2 Newton steps (no ACT tables).
                    i32 = mybir.dt.int32
                    nc.vector.tensor_scalar(
                        T1[:].bitcast(i32), ps_sq[:].bitcast(i32), 1, None,
                        op0=ALU.logical_shift_right)
                    nc.vector.tensor_tensor(
                        RSQ[:].bitcast(i32), MAG[:].broadcast_to([128, 256]),
                        T1[:].bitcast(i32), op=ALU.subtract)
                    nc.vector.tensor_scalar(LNV[:], ps_sq[:], -0.5,
                                            -0.5 * EPS, op0=ALU.mult,
                                            op1=ALU.add)
                    for _ in range(p["sq_newton"]):
                        nc.vector.tensor_mul(T1[:], RSQ[:], RSQ[:])
                        nc.vector.tensor_mul(T1[:], LNV[:], T1[:])
                        nc.vector.tensor_scalar_add(T1[:], T1[:], 1.5)
                        nc.vector.tensor_mul(RSQ[:], RSQ[:], T1[:])
                    nc.vector.tensor_scalar_add(SSP1[:], ps_sq[:], 1.0)
                    nc.vector.reciprocal(RCP1[:], SSP1[:])
                    nc.vector.tensor_mul(T1[:], ps_sq[:], RCP1[:])
                    nc.vector.tensor_mul(SCL[:], T1[:], RSQ[:])
                else:
                    # rsqrt via exp(-0.5*ln)
                    nc.vector.tensor_scalar_add(SSP1[:], ps_sq[:], 1.0)
                    nc.vector.reciprocal(RCP1[:], SSP1[:])
                    nc.vector.tensor_scalar_add(LNV[:], ps_sq[:], EPS)
                    nc.scalar.activation(LNV[:], LNV[:], AF.Ln)
                    nc.scalar.activation(RSQ[:], LNV[:], AF.Exp, scale=-0.5)
                    nc.vector.tensor_mul(T1[:], RCP1[:], RSQ[:])
                    nc.vector.tensor_mul(SCL[:], ps_sq[:], T1[:])
                if t < 3:
                    nc.vector.tensor_mul(VSB[:], SSB[:], SCL[:])
                    vsb_b = (VSB[:].rearrange("q (oc b) -> q oc b", oc=4)
                             .unsqueeze(2).broadcast_to([128, 4, 8, 64]))
                    mbd4 = MBD[:].rearrange("q (oc o8 b) -> q oc o8 b",
                                            oc=4, o8=8)
                    vbd4 = VBD[:].rearrange("q (oc o8 b) -> q oc o8 b",
                                            oc=4, o8=8)
                    nc.vector.tensor_mul(vbd4, vsb_b, mbd4)
                else:
                    nc.vector.tensor_mul(VSF[:], SSB[:], SCL[:])
                    nc.sync.dma_start(vout_d[:], VSF[:])

            def drain(eng_ch, dst, src):
                if wscale != 1.0:
                    if eng_ch == "A":
                        nc.scalar.mul(dst, src, 1.0 / wscale)
                    elif eng_ch == "D":
                        nc.vector.tensor_scalar_mul(dst, src, 1.0 / wscale)
                    else:
                        nc.gpsimd.tensor_scalar_mul(dst, src, 1.0 / wscale)
                else:
                    if eng_ch == "A":
                        nc.scalar.copy(dst, src)
                    elif eng_ch == "D":
                        nc.vector.tensor_copy(dst, src)
                    else:
                        nc.gpsimd.tensor_copy(dst, src)

            def eng_of(ch):
                return nc.vector if ch == "D" else nc.gpsimd

            for rep in range(reps):
                # ---------- iteration 1: s1 = (1/32) sum_i u_hat ----------
                # NOTE: each oc accumulation group needs its own PSUM bank
                # (start=True resets per-bank) -> oc*512 in a 4-bank tile
                ps_s1 = ps_pool_s.tile([128, 2048], f32, tag="ps_s",
                                       name=f"ps_s0_{rep}")
                for Gq in range(NG):
                    for k in range(8):
                        for oc in range(4):
                            nc.tensor.matmul(
                                ps_s1[:, oc * 512 : oc * 512 + 64],
                                WBs((Gq * 8 + k) * 4 + oc),
                                UY[:, (Gq * 8 + k) * 64 :
                                      (Gq * 8 + k) * 64 + 64],
                                start=(Gq == 0 and k == 0),
                                stop=(Gq == NG - 1 and k == 7),
                            )
                for oc in range(4):
                    nc.scalar.mul(SSB[:, oc * 64 : oc * 64 + 64],
                                  ps_s1[:, oc * 512 : oc * 512 + 64],
                                  1.0 / (32.0 * wscale))
                squash(1, rep)

                # ---------- iterations 2, 3 ----------
                for t in (2, 3):
                    ps_s = ps_pool_s.tile([128, 2048], f32, tag="ps_s",
                                          name=f"ps_s{rep}_{t}")

                    def wv_half(Gq, ks):
                        WVG = WVP[Gq % 2]
                        for k in ks:
                            for ocp in range(2) if "skipwv" not in vset else []:
                                wvp = ps_pool_wv.tile(
                                    [128, 1024], f32, tag="wvp",
                                    name=f"wvp_{rep}_{t}_{Gq}_{k}_{ocp}")
                                for kk in range(2):
                                    oc = ocp * 2 + kk
                                    nc.tensor.matmul(
                                        wvp[:, kk * 512 : kk * 512 + 512],
                                        WD[:, ((oc * 4 + Gq) * 8 + k) * 128 :
                                              ((oc * 4 + Gq) * 8 + k) * 128
                                              + 128],
                                        VBD[:, oc * 512 : oc * 512 + 512],
                                        start=True, stop=True,
                                    )
                                dst = WVG[:, k * 2048 + ocp * 1024 :
                                             k * 2048 + ocp * 1024 + 1024]
                                dplan = (p["drain_g0"] if Gq == 0
                                         else p["drain"])
                                drain(dplan[k * 2 + ocp], dst, wvp[:])

                    def post_a(Gq):
                        WVG = WVP[Gq % 2]
                        EG = EGP[Gq % 2]
                        wvg4 = WVG[:].rearrange("p (k x b) -> p k x b",
                                                k=8, x=32)
                        uyg4 = (UY[:, Gq * 512 : Gq * 512 + 512]
                                .rearrange("p (k b) -> p k b", k=8)
                                .unsqueeze(2).broadcast_to([128, 8, 32, 64]))
                        # k-slab pair view for the strided halving tree:
                        # [p, kp, two, x] with two = adjacent k-slabs
                        wpair = WVG[:].rearrange(
                            "p (kp two x) -> p kp two x", two=2, x=2048)
                        npk = p["amul_pool"]  # 0 or 2 tail ks on Pool
                        dk = 8 - npk
                        if "skipmt" not in vset:
                            # Pool takes the tail ks (slow engine, issued
                            # first so it overlaps the DVE halves)
                            if npk:
                                nc.gpsimd.tensor_mul(
                                    wvg4[:, dk:8], wvg4[:, dk:8],
                                    uyg4[:, dk:8])
                                nc.gpsimd.tensor_add(
                                    wpair[:, dk // 2 : 4, 0],
                                    wpair[:, dk // 2 : 4, 0],
                                    wpair[:, dk // 2 : 4, 1])
                            # a-path mul in two k-halves (chases the drains),
                            # paired k-sum tree right behind each half
                            nc.vector.tensor_mul(
                                wvg4[:, 0:4], wvg4[:, 0:4], uyg4[:, 0:4])
                            nc.vector.tensor_add(
                                wpair[:, 0:2, 0], wpair[:, 0:2, 0],
                                wpair[:, 0:2, 1])
                            if dk > 4:
                                nc.vector.tensor_mul(
                                    wvg4[:, 4:dk], wvg4[:, 4:dk],
                                    uyg4[:, 4:dk])
                                nc.vector.tensor_add(
                                    wpair[:, 2 : dk // 2, 0],
                                    wpair[:, 2 : dk // 2, 0],
                                    wpair[:, 2 : dk // 2, 1])
                            # quads: (k01)+=(k23) at slabs 0,8192; +4096 src
                            wq = WVG[:].rearrange(
                                "p (kq rest) -> p kq rest", kq=2)
                            nc.vector.tensor_add(
                                wq[:, :, 0:2048], wq[:, :, 0:2048],
                                wq[:, :, 4096:6144])
                        gsl = slice(Gq * 2048, Gq * 2048 + 2048)
                        if t == 2:
                            # final tree add lands directly in BLOG
                            nc.vector.tensor_add(
                                BLOG[:, gsl], WVG[:, 0:2048],
                                WVG[:, 8192:10240])
                            nc.scalar.activation(EG[:], BLOG[:, gsl], AF.Exp)
                        else:
                            nc.vector.tensor_add(
                                AG[:], WVG[:, 0:2048], WVG[:, 8192:10240])
                            nc.vector.tensor_add(AG[:], BLOG[:, gsl], AG[:])
                            nc.scalar.activation(EG[:], AG[:], AF.Exp)

                    def post_b(Gq):
                        WVG = WVP[Gq % 2]
                        EG = EGP[Gq % 2]
                        URC = URCP[Gq % 2]
                        wvg4 = WVG[:].rearrange("p (k x b) -> p k x b",
                                                k=8, x=32)
                        # sum over o: contiguous halving tree (o is outer)
                        nc.vector.tensor_add(SMT[:], EG[:, 0:1024],
                                             EG[:, 1024:2048])
                        nc.vector.tensor_add(SMT[:, 0:512], SMT[:, 0:512],
                                             SMT[:, 512:1024])
                        nc.vector.tensor_add(SMT[:, 0:256], SMT[:, 0:256],
                                             SMT[:, 256:512])
                        nc.vector.tensor_add(SMT[:, 0:128], SMT[:, 0:128],
                                             SMT[:, 128:256])
                        nc.vector.tensor_add(SE[:], SMT[:, 0:64],
                                             SMT[:, 64:128])
                        nc.vector.reciprocal(RCP[:], SE[:])
                        # urc = u * (1/Z): folds softmax denom into cu
                        # (RCP stays f32; avoids an ACT round-trip)
                        nc.vector.tensor_mul(
                            URC[:].rearrange("p (k b) -> p k b", k=8),
                            UY[:, Gq * 512 : Gq * 512 + 512]
                               .rearrange("p (k b) -> p k b", k=8),
                            RCP[:].unsqueeze(1).broadcast_to([128, 8, 64]))
                        # cu = e * urc (into WVG, now dead), built per-k so
                        # the s-path matmuls chase each chunk on PE
                        egb4 = (EG[:].rearrange("p (x b) -> p x b", x=32)
                                .unsqueeze(1).broadcast_to([128, 8, 32, 64]))
                        urc4 = (URC[:].rearrange("p (k b) -> p k b", k=8)
                                .unsqueeze(2).broadcast_to([128, 8, 32, 64]))
                        cks = set(p["cu_pool_ks"] if Gq < NG - 1
                                  else p["cu_pool_ks_last"])
                        # Pool chunks issued first (slow engine, runs in
                        # parallel with DVE's chunks)
                        if "skipcu" not in vset:
                            for k in sorted(cks):
                                nc.gpsimd.tensor_mul(
                                    wvg4[:, k:k+1], egb4[:, k:k+1],
                                    urc4[:, k:k+1])
                        def s_mm(k):
                            for oc in range(4):
                                nc.tensor.matmul(
                                    ps_s[:, oc * 512 : oc * 512 + 512],
                                    WBs((Gq * 8 + k) * 4 + oc),
                                    WVG[:, k * 2048 + oc * 512 :
                                           k * 2048 + oc * 512 + 512],
                                    start=(Gq == 0 and k == 0),
                                    stop=(Gq == NG - 1 and k == 7),
                                )
                        dks = [k for k in range(8) if k not in cks]
                        for k in dks:
                            if "skipcu" not in vset:
                                nc.vector.tensor_mul(
                                    wvg4[:, k:k+1], egb4[:, k:k+1],
                                    urc4[:, k:k+1])
                            if "skipsmm" not in vset:
                                s_mm(k)
                        if "skipsmm" not in vset:
                            for k in sorted(cks):
                                s_mm(k)

                    # software pipeline: wv(G) is split in halves with
                    # post_a(G-1) issued between them so exp(G-1) sits
                    # behind only 8 drains in the ACT FIFO, not 16.
                    for Gq in range(NG + 1):
                        if Gq < NG:
                            wv_half(Gq, range(0, 4))
                        if Gq >= 1:
                            post_a(Gq - 1)
                        if Gq < NG:
                            wv_half(Gq, range(4, 8))
                        if Gq >= 1:
                            post_b(Gq - 1)
                    # fused diag extract via mask
                    if "skipsmm" not in vset:
                        for oc in range(4):
                            nc.vector.tensor_mul(
                                MSKD[:], ps_s[:, oc * 512 : oc * 512 + 512],
                                MSK[:])
                            nc.vector.tensor_reduce(
                                SSB[:, oc * 64 : oc * 64 + 64],
                                MSKD[:].rearrange("q (o8 b) -> q b o8", o8=8),
                                axis=mybir.AxisListType.X, op=ALU.add)
                    squash(t, rep)

    nc.compile()
    return nc


def _host_prep(u, W, plan=None):
    """Build per-core input maps (all host-side permutes)."""
    import concourse.mybir as mybir

    p = dict(PLAN)
    if plan:
        p.update(plan)
    wnp = mybir.dt.np(mybir.dt.float8e4) if p["wfp8"] else _BF16
    wscale = WS if p["wfp8"] else 1.0

    in_maps = []
    q = np.arange(128)
    dlt = (q[:, None] // 16 == q[None, :] // 16).astype(_BF16)
    o8p = np.arange(8)
    diag = (q[:, None] // 16 == o8p[None, :])
    mbd = np.ascontiguousarray(
        np.broadcast_to(diag[:, None, :, None], (128, 4, 8, 64))
    ).reshape(128, 2048).astype(_BF16)
    msk = np.ascontiguousarray(
        np.broadcast_to(diag[:, :, None], (128, 8, 64))
    ).reshape(128, 512).astype(np.float32) / wscale
    for c in range(N_CORES):
        Ws = np.asarray(W[c * I_LOC : (c + 1) * I_LOC], dtype=np.float32)
        Ws = Ws * wscale
        us = np.asarray(u[:, c * I_LOC : (c + 1) * I_LOC, :], dtype=np.float32)
        Wr = Ws.reshape(NG, 128, 4, 8, 16, 8)           # [Gq,p,oc,o8,j,k]
        wd = np.ascontiguousarray(
            Wr.transpose(3, 4, 2, 0, 5, 1)              # [o8,j,oc,Gq,k,p]
        ).reshape(128, 16384).astype(wnp)
        wb = np.ascontiguousarray(
            Wr.transpose(1, 0, 5, 2, 3, 4)              # [p,Gq,k,oc,o8,j]
        ).reshape(128, 16384).astype(wnp)
        ur = us.reshape(B, NG, 128, 8)                  # [b,Gq,p,k]
        uy = np.ascontiguousarray(
            ur.transpose(2, 1, 3, 0)                    # [p,Gq,k,b]
        ).reshape(128, 2048).astype(_BF16)
        in_maps.append({"wd": wd, "wb": wb, "uy": uy, "dlt": dlt,
                        "mbd": mbd, "msk": msk})
    return in_maps


def kernel(u, W):
    from concourse.bass_utils import run_bass_kernel_spmd

    if "nc" not in _CACHE:
        _CACHE["nc"] = _build_program(variant="ag")
    nc = _CACHE["nc"]

    in_maps = _host_prep(u, W)
    res = run_bass_kernel_spmd(
        nc, in_maps, core_ids=list(range(N_CORES)),
        trace=bool(int(os.environ.get("CAPS_TRACE", "0"))),
    )
    if isinstance(res, tuple):
        results = res[0]
    else:
        _CACHE["last_results"] = res
        results = res.results
    vout = results[0]["vout"]  # [128, 256]; identical on every core
    t = vout.reshape(8, 16, 4, 64)          # [o8, j, oc, b]
    v = np.ascontiguousarray(t.transpose(3, 2, 0, 1)).reshape(B, OUT_CAPS, OUT_DIM)
    return v.astype(np.float32)



# revision 20
# speedup vs baseline: 1.0603x; 1.0048x over previous
"""CapsNet routing-by-agreement kernel for 8 TRN2 NeuronCores.

Strategy (in_caps sharded 8-way):
  - Each core owns I_loc = 512 in_caps. Its W shard lives entirely in SBUF
    (two layouts), so routing iterations do ZERO HBM traffic for W / u_hat.
    u_hat is never materialized; each routing iteration recomputes the two
    W contractions on the TensorEngine with 128-deep packed contractions:
      a-path:  Wv[b,i,o,k] = sum_j W[i,o,j,k] v[b,o,j]   (contract (o8,j)=128,
               block-diag v as stationary operand)
               a[b,i,o]    = sum_k u[b,i,k] Wv[b,i,o,k]  (DVE mul + add-tree)
      s-path:  s[b,o,j]    = sum_{i,k} (c*u)[..] W[..]   (contract i mod 128,
               PSUM-accum over (i-block, k); 8x block-diag fanout over o8,
               diagonal extracted with a constant mask)
  - The only cross-core data is the per-out-capsule sum s [64,32,16]:
    AllGather (fp8 for routing iterations, bf16 for the final one) + local
    on-chip sum, once per routing iteration (3x).
  - Elementwise work is load-balanced across DVE / ACT / Pool(GPSIMD) via
    the PLAN knobs; PSUM drains rotate over all three engines.
  - All layout permutations of the inputs are done host-side in numpy; every
    on-chip tensor is DMA'd contiguously.

Index conventions (per core c): i_glob = c*512 + Gq*128 + p (Gq in 0..3,
p in 0..127);  o = oc*8 + o8 (oc in 0..3);  q = o8*16 + j (j in 0..15).

Host layouts:
  wd  [128,16384] : wd[q, ((oc*4+Gq)*8+k)*128 + p]  = W[i,o,j,k] * WS
  wb  [128,16384] : wb[p, ((Gq*8+k)*4+oc)*128 + q]  = W[i,o,j,k] * WS
  uy  [128, 2048] bf16 : uy[p, (Gq*8+k)*64 + b]     = u[b,i,k]
  dlt [128,  128] bf16 : dlt[q, m] = (q//16 == m//16)  (squash sum_j)
  mbd [128, 2048] bf16 : mbd[q, (oc,o8p,b)] = (q//16 == o8p)
  msk [128,  512] f32  : msk[q, (o8p,b)]   = (q//16 == o8p) / WS
Output:
  vout [128, 256] f32 : vout[o8*16+j, oc*64+b] = v[b, oc*8+o8, j]
"""

import json
import os
import sys

import numpy as np
import ml_dtypes

sys.path.insert(0, "/opt/trn_rl_repo")

B, IN_CAPS, IN_DIM = 64, 4096, 8
OUT_CAPS, OUT_DIM = 32, 16
EPS = 1e-8
N_CORES = 8
I_LOC = IN_CAPS // N_CORES  # 512
NG = I_LOC // 128           # 4

_BF16 = ml_dtypes.bfloat16

# Engine-assignment plan. Letters: A=ACT(scalar), D=DVE(vector), P=Pool(gpsimd)
PLAN = {
    # one char per wv-phase drain chunk, idx = k*2 + ocp (16 chunks of
    # [128,1024] f32 PSUM -> bf16 SBUF per Gq). Pool/GPSIMD cannot read
    # PSUM (BIR verifier), so only A/D are valid here.
    "drain": "AAAAAAAAAAAAAAAA",
    # drain schedule for the FIRST Gq of each iteration (measured: DVE
    # shares regress — ACT-only wins)
    "drain_g0": "AAAAAAAAAAAAAAAA",
    # which k's (0..7) of the cu build (c*u) go to Pool
    "cu_pool_ks": (5, 6, 7),
    # same, for the LAST Gq: its cu chunks gate s_mm -> exchange, so Pool
    # lag there extends the iteration tail
    "cu_pool_ks_last": (),
    # number of TAIL k's (0 or 2) of the a-path mul (Wv*u) + their pair
    # add that run on Pool instead of DVE
    "amul_pool": 0,
    # squash via fused tensor_scalar pow (vs Ln/Exp on ACT)
    "sq_pow": True,
    # newton refinement steps for the quake rsqrt seed (1 => ~0.17% max err)
    "sq_newton": 1,
    # fp8 payload for the routing AllGathers (iterations 1, 2).
    # Measured on HW: costs ~1.3e-2 of rel err — too close to the 2e-2
    # gate to enable by default.
    "ag_fp8": False,
    # W layouts in fp8e4 (scaled by WS), stationary-only. Measured on HW:
    # ~2.4e-2 rel err through routing feedback — keep off.
    "wfp8": False,
}
if os.environ.get("CAPS_PLAN"):
    PLAN.update(json.loads(os.environ["CAPS_PLAN"]))

WS = 16.0  # host-side W scale for fp8 dynamic range (descaled on chip)

_CACHE = {}


def _build_program(reps=1, variant="full", plan=None):
    import concourse.bass as bass
    import concourse.bacc as bacc
    import concourse.mybir as mybir
    import concourse.tile as tile
    from contextlib import ExitStack

    p = dict(PLAN)
    if plan:
        p.update(plan)

    f32 = mybir.dt.float32
    bf16 = mybir.dt.bfloat16
    fp8 = mybir.dt.float8e4
    wdt = fp8 if p["wfp8"] else bf16
    wscale = WS if p["wfp8"] else 1.0
    vset = set(variant.split(","))
    exch = ("ag" if "ag" in vset else
            "ar" if "ar" in vset else
            "fakeexch" if "fakeexch" in vset else
            "noexch" if "noexch" in vset else "ag")
    AF = mybir.ActivationFunctionType
    ALU = mybir.AluOpType

    nc = bacc.Bacc(
        "TRN2",
        target_bir_lowering=False,
        debug=False,
        enable_asserts=False,
        num_devices=N_CORES,
    )

    wd_d = nc.dram_tensor("wd", [128, 16384], wdt, kind="ExternalInput")
    wb_d = nc.dram_tensor("wb", [128, 16384], wdt, kind="ExternalInput")
    uy_d = nc.dram_tensor("uy", [128, 2048], bf16, kind="ExternalInput")
    dlt_d = nc.dram_tensor("dlt", [128, 128], bf16, kind="ExternalInput")
    mbd_d = nc.dram_tensor("mbd", [128, 2048], bf16, kind="ExternalInput")
    msk_d = nc.dram_tensor("msk", [128, 512], f32, kind="ExternalInput")
    vout_d = nc.dram_tensor("vout", [128, 256], f32, kind="ExternalOutput")

    with tile.TileContext(nc) as tc:
        with ExitStack() as ctx:
            sb = ctx.enter_context(tc.tile_pool(name="sb", bufs=1))
            ps_pool_s = ctx.enter_context(
                tc.tile_pool(name="ps_s", bufs=1, space="PSUM"))
            ps_pool_wv = ctx.enter_context(
                tc.tile_pool(name="ps_wv", bufs=2, space="PSUM"))
            dram = ctx.enter_context(tc.tile_pool(name="dram", bufs=2,
                                                  space="DRAM"))

            WD = sb.tile([128, 16384], wdt, tag="WD")
            WBH = [sb.tile([128, 8192], wdt, tag=f"WB{i}", name=f"WB{i}")
                   for i in range(2)]
            UY = sb.tile([128, 2048], bf16, tag="UY")
            DLT = sb.tile([128, 128], bf16, tag="DLT")
            MBD = sb.tile([128, 2048], bf16, tag="MBD")
            MSK = sb.tile([128, 512], f32, tag="MSK")

            # DMA order: iteration-1 needs WB halves + UY first; WD (a-path)
            # is only needed after the first AllGather; constants in between.
            nc.sync.dma_start(WBH[0][:], wb_d[:, 0:8192])
            nc.sync.dma_start(UY[:, 0:1024], uy_d[:, 0:1024])
            nc.sync.dma_start(WBH[1][:], wb_d[:, 8192:16384])
            nc.sync.dma_start(UY[:, 1024:2048], uy_d[:, 1024:2048])
            nc.sync.dma_start(DLT[:], dlt_d[:])
            nc.sync.dma_start(MBD[:], mbd_d[:])
            nc.sync.dma_start(MSK[:], msk_d[:])
            nc.sync.dma_start(WD[:], wd_d[:])

            def WBs(idx):
                half, off = divmod(idx * 128, 8192)
                return WBH[half][:, off : off + 128]

            # big per-G scratch, parity double-buffered:
            # wv (drains) -> uwv (in-place mul) -> tree partials -> cu
            WVP = [sb.tile([128, 16384], bf16, tag=f"WV{i}", name=f"WV{i}")
                   for i in range(2)]
            BLOG = sb.tile([128, 8192], bf16, tag="BLOG")
            AG = sb.tile([128, 2048], bf16, tag="AG")
            # EG parity-double-buffered so exp(G) overlaps post_b(G-1) reads
            EGP = [sb.tile([128, 2048], bf16, tag=f"EG{i}", name=f"EG{i}")
                   for i in range(2)]
            # URC parity-buffered: Pool's cu(G-1) reads must not block the
            # DVE write of URC(G)
            URCP = [sb.tile([128, 512], bf16, tag=f"URC{i}", name=f"URC{i}")
                    for i in range(2)]
            SMT = sb.tile([128, 1024], bf16, tag="SMT")
            MSKD = sb.tile([128, 512], f32, tag="MSKD")
            GAT = sb.tile([128, 2048], bf16, tag="GAT")
            GAT8 = sb.tile([128, 2048], fp8, tag="GAT8")
            GATB = sb.tile([128, 1024], bf16, tag="GATB")
            SSB = sb.tile([128, 256], f32, tag="SSB")
            SSBH = sb.tile([128, 256], bf16, tag="SSBH")
            SSB8 = sb.tile([128, 256], fp8, tag="SSB8")
            SE = sb.tile([128, 64], f32, tag="SE")
            RCP = sb.tile([128, 64], f32, tag="RCP")
            RCPB = sb.tile([128, 64], bf16, tag="RCPB")
            VBD = sb.tile([128, 2048], bf16, tag="VBD")
            S2 = sb.tile([128, 256], bf16, tag="S2")
            SSP1 = sb.tile([128, 256], f32, tag="SSP1")
            RCP1 = sb.tile([128, 256], f32, tag="RCP1")
            LNV = sb.tile([128, 256], f32, tag="LNV")
            RSQ = sb.tile([128, 256], f32, tag="RSQ")
            T1 = sb.tile([128, 256], f32, tag="T1")
            SCL = sb.tile([128, 256], f32, tag="SCL")
            VSB = sb.tile([128, 256], bf16, tag="VSB")
            VSF = sb.tile([128, 256], f32, tag="VSF")
            MAG = sb.tile([128, 1], mybir.dt.int32, tag="MAG")
            nc.vector.memset(MAG[:], 0x5F3759DF)

            def exchange(t, rep):
                """SSB (partial s) -> cross-core sum back into SSB."""
                use_fp8 = p["ag_fp8"] and t < 3 and exch == "ag"
                if exch == "fakeexch":
                    # timing probe: same DMAs + adds as "ag", but the
                    # collective is replaced by a local 8x DRAM broadcast
                    nc.vector.tensor_copy(SSBH[:], SSB[:])
                    ag_in = dram.tile([128, 256], bf16, tag="ag_in",
                                      name=f"ag_in_{rep}_{t}")
                    ag_out = dram.tile([1024, 256], bf16, tag="ag_out",
                                       name=f"ag_out_{rep}_{t}")
                    nc.sync.dma_start(ag_in[:], SSBH[:])
                    nc.sync.dma_start(
                        ag_out[:].rearrange("(r p) f -> r p f", r=8),
                        ag_in[:].unsqueeze(0).broadcast_to([8, 128, 256]),
                    )
                    nc.sync.dma_start(
                        GAT[:].rearrange("p (r f) -> p r f", r=8),
                        ag_out[:].rearrange("(r p) f -> p r f", p=128),
                    )
                    g3 = GAT[:].rearrange("p (r f) -> p r f", r=8)
                    nc.vector.tensor_add(g3[:, 0:4], g3[:, 0:4], g3[:, 4:8])
                    nc.vector.tensor_add(g3[:, 0:2], g3[:, 0:2], g3[:, 2:4])
                    nc.vector.tensor_add(SSB[:], GAT[:, 0:256],
                                         GAT[:, 256:512])
                    return
                if exch == "ag":
                    if use_fp8:
                        nc.vector.tensor_copy(SSB8[:], SSB[:])
                        ag_in = dram.tile([128, 256], fp8, tag="ag_in",
                                          name=f"ag_in_{rep}_{t}")
                        ag_out = dram.tile([1024, 256], fp8, tag="ag_out",
                                           name=f"ag_out_{rep}_{t}")
                        nc.sync.dma_start(ag_in[:], SSB8[:])
                    else:
                        nc.vector.tensor_copy(SSBH[:], SSB[:])
                        ag_in = dram.tile([128, 256], bf16, tag="ag_in",
                                          name=f"ag_in_{rep}_{t}")
                        ag_out = dram.tile([1024, 256], bf16, tag="ag_out",
                                           name=f"ag_out_{rep}_{t}")
                        nc.sync.dma_start(ag_in[:], SSBH[:])
                    nc.gpsimd.collective_compute(
                        "AllGather", ALU.bypass,
                        replica_groups=[list(range(N_CORES))],
                        ins=[ag_in[:].opt()], outs=[ag_out[:].opt()],
                    )
                    if use_fp8:
                        nc.sync.dma_start(
                            GAT8[:].rearrange("p (r f) -> p r f", r=8),
                            ag_out[:].rearrange("(r p) f -> p r f", p=128),
                        )
                        nc.vector.tensor_add(GATB[:], GAT8[:, 0:1024],
                                             GAT8[:, 1024:2048])
                        nc.vector.tensor_add(GATB[:, 0:512], GATB[:, 0:512],
                                             GATB[:, 512:1024])
                        nc.vector.tensor_add(SSB[:], GATB[:, 0:256],
                                             GATB[:, 256:512])
                    else:
                        nc.sync.dma_start(
                            GAT[:].rearrange("p (r f) -> p r f", r=8),
                            ag_out[:].rearrange("(r p) f -> p r f", p=128),
                        )
                        g3 = GAT[:].rearrange("p (r f) -> p r f", r=8)
                        nc.vector.tensor_add(g3[:, 0:4], g3[:, 0:4], g3[:, 4:8])
                        nc.vector.tensor_add(g3[:, 0:2], g3[:, 0:2], g3[:, 2:4])
                        nc.vector.tensor_add(SSB[:], GAT[:, 0:256],
                                             GAT[:, 256:512])
                elif exch == "ar":
                    ar_in = dram.tile([128, 256], f32, tag="ar_in",
                                      name=f"ar_in_{rep}_{t}")
                    ar_out = dram.tile([128, 256], f32, tag="ar_out",
                                       name=f"ar_out_{rep}_{t}")
                    nc.gpsimd.dma_start(ar_in[:], SSB[:])
                    nc.gpsimd.collective_compute(
                        "AllReduce", ALU.add,
                        replica_groups=[list(range(N_CORES))],
                        ins=[ar_in[:].opt()], outs=[ar_out[:].opt()],
                    )
                    nc.gpsimd.dma_start(SSB[:], ar_out[:])
                elif exch == "noexch":
                    pass  # timing-only: skip the cross-core exchange

            def squash(t, rep):
                exchange(t, rep)
                # ss = sum_j s^2 (dup'd over 16 via DLT matmul)
                nc.vector.tensor_mul(S2[:], SSB[:], SSB[:])
                ps_sq = ps_pool_wv.tile([128, 256], f32, tag="wvp",
                                        name=f"ps_sq_{rep}_{t}")
                nc.tensor.matmul(ps_sq[:], DLT[:], S2[:], start=True, stop=True)
                if p["sq_pow"]:
                    # scale = (ss/(1+ss)) * rsqrt(ss+eps), all on DVE.
                    # rsqrt via quake seed + 2 Newton steps (no ACT tables).
                    i32 = mybir.dt.int32
                    nc.vector.tensor_scalar(
                        T1[:].bitcast(i32), ps_sq[:].bitcast(i32), 1, None,
                        op0=ALU.logical_shift_right)
                    nc.vector.tensor_tensor(
                        RSQ[:].bitcast(i32), MAG[:].broadcast_to([128, 256]),
                        T1[:].bitcast(i32), op=ALU.subtract)
                    nc.vector.tensor_scalar(LNV[:], ps_sq[:], -0.5,
                                            -0.5 * EPS, op0=ALU.mult,
                                            op1=ALU.add)
                    for _ in range(p["sq_newton"]):
                        nc.vector.tensor_mul(T1[:], RSQ[:], RSQ[:])
                        nc.vector.tensor_mul(T1[:], LNV[:], T1[:])
                        nc.vector.tensor_scalar_add(T1[:], T1[:], 1.5)
                        nc.vector.tensor_mul(RSQ[:], RSQ[:], T1[:])
                    nc.vector.tensor_scalar_add(SSP1[:], ps_sq[:], 1.0)
                    nc.vector.reciprocal(RCP1[:], SSP1[:])
                    nc.vector.tensor_mul(T1[:], ps_sq[:], RCP1[:])
                    nc.vector.tensor_mul(SCL[:], T1[:], RSQ[:])
                else:
                    # rsqrt via exp(-0.5*ln)
                    nc.vector.tensor_scalar_add(SSP1[:], ps_sq[:], 1.0)
                    nc.vector.reciprocal(RCP1[:], SSP1[:])
                    nc.vector.tensor_scalar_add(LNV[:], ps_sq[:], EPS)
                    nc.scalar.activation(LNV[:], LNV[:], AF.Ln)
                    nc.scalar.activation(RSQ[:], LNV[:], AF.Exp, scale=-0.5)
                    nc.vector.tensor_mul(T1[:], RCP1[:], RSQ[:])
                    nc.vector.tensor_mul(SCL[:], ps_sq[:], T1[:])
                if t < 3:
                    nc.vector.tensor_mul(VSB[:], SSB[:], SCL[:])
                    vsb_b = (VSB[:].rearrange("q (oc b) -> q oc b", oc=4)
                             .unsqueeze(2).broadcast_to([128, 4, 8, 64]))
                    mbd4 = MBD[:].rearrange("q (oc o8 b) -> q oc o8 b",
                                            oc=4, o8=8)
                    vbd4 = VBD[:].rearrange("q (oc o8 b) -> q oc o8 b",
                                            oc=4, o8=8)
                    nc.vector.tensor_mul(vbd4, vsb_b, mbd4)
                else:
                    nc.vector.tensor_mul(VSF[:], SSB[:], SCL[:])
                    nc.sync.dma_start(vout_d[:], VSF[:])

            def drain(eng_ch, dst, src):
                if wscale != 1.0:
                    if eng_ch == "A":
                        nc.scalar.mul(dst, src, 1.0 / wscale)
                    elif eng_ch == "D":
                        nc.vector.tensor_scalar_mul(dst, src, 1.0 / wscale)
                    else:
                        nc.gpsimd.tensor_scalar_mul(dst, src, 1.0 / wscale)
                else:
                    if eng_ch == "A":
                        nc.scalar.copy(dst, src)
                    elif eng_ch == "D":
                        nc.vector.tensor_copy(dst, src)
                    else:
                        nc.gpsimd.tensor_copy(dst, src)

            def eng_of(ch):
                return nc.vector if ch == "D" else nc.gpsimd

            for rep in range(reps):
                # ---------- iteration 1: s1 = (1/32) sum_i u_hat ----------
                # psum from the wv pool (free at the prior rep's tail) so
                # these matmuls overlap squash(3)/vout of the previous rep.
                # Each oc accumulation group gets its own PSUM bank
                # (start=True resets per-bank): oc pairs at offsets 0/512
                # of two [128,1024] (2-bank) tiles.
                ps_s1 = [ps_pool_wv.tile([128, 1024], f32, tag="wvp",
                                         name=f"ps_s0_{rep}_{h}")
                         for h in range(2)]

                def s1_slot(oc):
                    return ps_s1[oc // 2][:, (oc % 2) * 512 :
                                          (oc % 2) * 512 + 64]
                for Gq in range(NG):
                    for k in range(8):
                        for oc in range(4):
                            nc.tensor.matmul(
                                s1_slot(oc),
                                WBs((Gq * 8 + k) * 4 + oc),
                                UY[:, (Gq * 8 + k) * 64 :
                                      (Gq * 8 + k) * 64 + 64],
                                start=(Gq == 0 and k == 0),
                                stop=(Gq == NG - 1 and k == 7),
                            )
                for oc in range(4):
                    nc.scalar.mul(SSB[:, oc * 64 : oc * 64 + 64],
                                  s1_slot(oc), 1.0 / (32.0 * wscale))
                squash(1, rep)

                # ---------- iterations 2, 3 ----------
                for t in (2, 3):
                    ps_s = ps_pool_s.tile([128, 2048], f32, tag="ps_s",
                                          name=f"ps_s{rep}_{t}")

                    def wv_half(Gq, ks):
                        WVG = WVP[Gq % 2]
                        for k in ks:
                            for ocp in range(2) if "skipwv" not in vset else []:
                                wvp = ps_pool_wv.tile(
                                    [128, 1024], f32, tag="wvp",
                                    name=f"wvp_{rep}_{t}_{Gq}_{k}_{ocp}")
                                for kk in range(2):
                                    oc = ocp * 2 + kk
                                    nc.tensor.matmul(
                                        wvp[:, kk * 512 : kk * 512 + 512],
                                        WD[:, ((oc * 4 + Gq) * 8 + k) * 128 :
                                              ((oc * 4 + Gq) * 8 + k) * 128
                                              + 128],
                                        VBD[:, oc * 512 : oc * 512 + 512],
                                        start=True, stop=True,
                                    )
                                dst = WVG[:, k * 2048 + ocp * 1024 :
                                             k * 2048 + ocp * 1024 + 1024]
                                dplan = (p["drain_g0"] if Gq == 0
                                         else p["drain"])
                                drain(dplan[k * 2 + ocp], dst, wvp[:])

                    def post_a(Gq):
                        WVG = WVP[Gq % 2]
                        EG = EGP[Gq % 2]
                        wvg4 = WVG[:].rearrange("p (k x b) -> p k x b",
                                                k=8, x=32)
                        uyg4 = (UY[:, Gq * 512 : Gq * 512 + 512]
                                .rearrange("p (k b) -> p k b", k=8)
                                .unsqueeze(2).broadcast_to([128, 8, 32, 64]))
                        # k-slab pair view for the strided halving tree:
                        # [p, kp, two, x] with two = adjacent k-slabs
                        wpair = WVG[:].rearrange(
                            "p (kp two x) -> p kp two x", two=2, x=2048)
                        npk = p["amul_pool"]  # 0 or 2 tail ks on Pool
                        dk = 8 - npk
                        if "skipmt" not in vset:
                            # Pool takes the tail ks (slow engine, issued
                            # first so it overlaps the DVE halves)
                            if npk:
                                nc.gpsimd.tensor_mul(
                                    wvg4[:, dk:8], wvg4[:, dk:8],
                                    uyg4[:, dk:8])
                                nc.gpsimd.tensor_add(
                                    wpair[:, dk // 2 : 4, 0],
                                    wpair[:, dk // 2 : 4, 0],
                                    wpair[:, dk // 2 : 4, 1])
                            # a-path mul in two k-halves (chases the drains),
                            # paired k-sum tree right behind each half
                            nc.vector.tensor_mul(
                                wvg4[:, 0:4], wvg4[:, 0:4], uyg4[:, 0:4])
                            nc.vector.tensor_add(
                                wpair[:, 0:2, 0], wpair[:, 0:2, 0],
                                wpair[:, 0:2, 1])
                            if dk > 4:
                                nc.vector.tensor_mul(
                                    wvg4[:, 4:dk], wvg4[:, 4:dk],
                                    uyg4[:, 4:dk])
                                nc.vector.tensor_add(
                                    wpair[:, 2 : dk // 2, 0],
                                    wpair[:, 2 : dk // 2, 0],
                                    wpair[:, 2 : dk // 2, 1])
                            # quads: (k01)+=(k23) at slabs 0,8192; +4096 src
                            wq = WVG[:].rearrange(
                                "p (kq rest) -> p kq rest", kq=2)
                            nc.vector.tensor_add(
                                wq[:, :, 0:2048], wq[:, :, 0:2048],
                                wq[:, :, 4096:6144])
                        gsl = slice(Gq * 2048, Gq * 2048 + 2048)
                        if t == 2:
                            # final tree add lands directly in BLOG
                            nc.vector.tensor_add(
                                BLOG[:, gsl], WVG[:, 0:2048],
                                WVG[:, 8192:10240])
                            nc.scalar.activation(EG[:], BLOG[:, gsl], AF.Exp)
                        else:
                            nc.vector.tensor_add(
                                AG[:], WVG[:, 0:2048], WVG[:, 8192:10240])
                            nc.vector.tensor_add(AG[:], BLOG[:, gsl], AG[:])
                            nc.scalar.activation(EG[:], AG[:], AF.Exp)

                    def post_b(Gq):
                        WVG = WVP[Gq % 2]
                        EG = EGP[Gq % 2]
                        URC = URCP[Gq % 2]
                        wvg4 = WVG[:].rearrange("p (k x b) -> p k x b",
                                                k=8, x=32)
                        # sum over o: contiguous halving tree (o is outer)
                        nc.vector.tensor_add(SMT[:], EG[:, 0:1024],
                                             EG[:, 1024:2048])
                        nc.vector.tensor_add(SMT[:, 0:512], SMT[:, 0:512],
                                             SMT[:, 512:1024])
                        nc.vector.tensor_add(SMT[:, 0:256], SMT[:, 0:256],
                                             SMT[:, 256:512])
                        nc.vector.tensor_add(SMT[:, 0:128], SMT[:, 0:128],
                                             SMT[:, 128:256])
                        nc.vector.tensor_add(SE[:], SMT[:, 0:64],
                                             SMT[:, 64:128])
                        nc.vector.reciprocal(RCP[:], SE[:])
                        # urc = u * (1/Z): folds softmax denom into cu
                        # (RCP stays f32; avoids an ACT round-trip)
                        nc.vector.tensor_mul(
                            URC[:].rearrange("p (k b) -> p k b", k=8),
                            UY[:, Gq * 512 : Gq * 512 + 512]
                               .rearrange("p (k b) -> p k b", k=8),
                            RCP[:].unsqueeze(1).broadcast_to([128, 8, 64]))
                        # cu = e * urc (into WVG, now dead), built per-k so
                        # the s-path matmuls chase each chunk on PE
                        egb4 = (EG[:].rearrange("p (x b) -> p x b", x=32)
                                .unsqueeze(1).broadcast_to([128, 8, 32, 64]))
                        urc4 = (URC[:].rearrange("p (k b) -> p k b", k=8)
                                .unsqueeze(2).broadcast_to([128, 8, 32, 64]))
                        cks = set(p["cu_pool_ks"] if Gq < NG - 1
                                  else p["cu_pool_ks_last"])
                        # Pool chunks issued first (slow engine, runs in
                        # parallel with DVE's chunks)
                        if "skipcu" not in vset:
                            for k in sorted(cks):
                                nc.gpsimd.tensor_mul(
                                    wvg4[:, k:k+1], egb4[:, k:k+1],
                                    urc4[:, k:k+1])
                        def s_mm(k):
                            for oc in range(4):
                                nc.tensor.matmul(
                                    ps_s[:, oc * 512 : oc * 512 + 512],
                                    WBs((Gq * 8 + k) * 4 + oc),
                                    WVG[:, k * 2048 + oc * 512 :
                                           k * 2048 + oc * 512 + 512],
                                    start=(Gq == 0 and k == 0),
                                    stop=(Gq == NG - 1 and k == 7),
                                )
                        dks = [k for k in range(8) if k not in cks]
                        for k in dks:
                            if "skipcu" not in vset:
                                nc.vector.tensor_mul(
                                    wvg4[:, k:k+1], egb4[:, k:k+1],
                                    urc4[:, k:k+1])
                            if "skipsmm" not in vset:
                                s_mm(k)
                        if "skipsmm" not in vset:
                            for k in sorted(cks):
                                s_mm(k)

                    # software pipeline: wv(G) is split in halves with
                    # post_a(G-1) issued between them so exp(G-1) sits
                    # behind only 8 drains in the ACT FIFO, not 16.
                    for Gq in range(NG + 1):
                        if Gq < NG:
                            wv_half(Gq, range(0, 4))
                        if Gq >= 1:
                            post_a(Gq - 1)
                        if Gq < NG:
                            wv_half(Gq, range(4, 8))
                        if Gq >= 1:
                            post_b(Gq - 1)
                    # fused diag extract via mask
                    if "skipsmm" not in vset:
                        for oc in range(4):
                            nc.vector.tensor_mul(
                                MSKD[:], ps_s[:, oc * 512 : oc * 512 + 512],
                                MSK[:])
                            nc.vector.tensor_reduce(
                                SSB[:, oc * 64 : oc * 64 + 64],
                                MSKD[:].rearrange("q (o8 b) -> q b o8", o8=8),
                                axis=mybir.AxisListType.X, op=ALU.add)
                    squash(t, rep)

    nc.compile()
    return nc


def _host_prep(u, W, plan=None):
    """Build per-core input maps (all host-side permutes)."""
    import concourse.mybir as mybir

    p = dict(PLAN)
    if plan:
        p.update(plan)
    wnp = mybir.dt.np(mybir.dt.float8e4) if p["wfp8"] else _BF16
    wscale = WS if p["wfp8"] else 1.0

    in_maps = []
    q = np.arange(128)
    dlt = (q[:, None] // 16 == q[None, :] // 16).astype(_BF16)
    o8p = np.arange(8)
    diag = (q[:, None] // 16 == o8p[None, :])
    mbd = np.ascontiguousarray(
        np.broadcast_to(diag[:, None, :, None], (128, 4, 8, 64))
    ).reshape(128, 2048).astype(_BF16)
    msk = np.ascontiguousarray(
        np.broadcast_to(diag[:, :, None], (128, 8, 64))
    ).reshape(128, 512).astype(np.float32) / wscale
    for c in range(N_CORES):
        Ws = np.asarray(W[c * I_LOC : (c + 1) * I_LOC], dtype=np.float32)
        Ws = Ws * wscale
        us = np.asarray(u[:, c * I_LOC : (c + 1) * I_LOC, :], dtype=np.float32)
        Wr = Ws.reshape(NG, 128, 4, 8, 16, 8)           # [Gq,p,oc,o8,j,k]
        wd = np.ascontiguousarray(
            Wr.transpose(3, 4, 2, 0, 5, 1)              # [o8,j,oc,Gq,k,p]
        ).reshape(128, 16384).astype(wnp)
        wb = np.ascontiguousarray(
            Wr.transpose(1, 0, 5, 2, 3, 4)              # [p,Gq,k,oc,o8,j]
        ).reshape(128, 16384).astype(wnp)
        ur = us.reshape(B, NG, 128, 8)                  # [b,Gq,p,k]
        uy = np.ascontiguousarray(
            ur.transpose(2, 1, 3, 0)                    # [p,Gq,k,b]
        ).reshape(128, 2048).astype(_BF16)
        in_maps.append({"wd": wd, "wb": wb, "uy": uy, "dlt": dlt,
                        "mbd": mbd, "msk": msk})
    return in_maps


def kernel(u, W):
    from concourse.bass_utils import run_bass_kernel_spmd

    if "nc" not in _CACHE:
        _CACHE["nc"] = _build_program(variant="ag")
    nc = _CACHE["nc"]

    in_maps = _host_prep(u, W)
    res = run_bass_kernel_spmd(
        nc, in_maps, core_ids=list(range(N_CORES)),
        trace=bool(int(os.environ.get("CAPS_TRACE", "0"))),
    )
    if isinstance(res, tuple):
        results = res[0]
    else:
        _CACHE["last_results"] = res
        results = res.results
    vout = results[0]["vout"]  # [128, 256]; identical on every core
    t = vout.reshape(8, 16, 4, 64)          # [o8, j, oc, b]
    v = np.ascontiguousarray(t.transpose(3, 2, 0, 1)).reshape(B, OUT_CAPS, OUT_DIM)
    return v.astype(np.float32)



# revision 24
# speedup vs baseline: 1.2154x; 1.1464x over previous
"""CapsNet routing-by-agreement kernel for 8 TRN2 NeuronCores.

Strategy (in_caps sharded 8-way):
  - Each core owns I_loc = 512 in_caps. Its W shard lives entirely in SBUF
    (two layouts), so routing iterations do ZERO HBM traffic for W / u_hat.
    u_hat is never materialized; each routing iteration recomputes the two
    W contractions on the TensorEngine with 128-deep packed contractions:
      a-path:  Wv[b,i,o,k] = sum_j W[i,o,j,k] v[b,o,j]   (contract (o8,j)=128,
               block-diag v as stationary operand)
               a[b,i,o]    = sum_k u[b,i,k] Wv[b,i,o,k]  (DVE mul + add-tree)
      s-path:  s[b,o,j]    = sum_{i,k} (c*u)[..] W[..]   (contract i mod 128,
               PSUM-accum over (i-block, k); 8x block-diag fanout over o8,
               diagonal extracted with a constant mask)
  - The only cross-core data is the per-out-capsule sum s [64,32,16]:
    AllGather (fp8 for routing iterations, bf16 for the final one) + local
    on-chip sum, once per routing iteration (3x).
  - Elementwise work is load-balanced across DVE / ACT / Pool(GPSIMD) via
    the PLAN knobs; PSUM drains rotate over all three engines.
  - All layout permutations of the inputs are done host-side in numpy; every
    on-chip tensor is DMA'd contiguously.

Index conventions (per core c): i_glob = c*512 + Gq*128 + p (Gq in 0..3,
p in 0..127);  o = oc*8 + o8 (oc in 0..3);  q = o8*16 + j (j in 0..15).

Host layouts:
  wd  [128,16384] : wd[q, ((oc*4+Gq)*8+k)*128 + p]  = W[i,o,j,k] * WS
  wb  [128,16384] : wb[p, ((Gq*8+k)*4+oc)*128 + q]  = W[i,o,j,k] * WS
  uy  [128, 2048] bf16 : uy[p, (Gq*8+k)*64 + b]     = u[b,i,k]
  dlt [128,  128] bf16 : dlt[q, m] = (q//16 == m//16)  (squash sum_j)
  mbd [128, 2048] bf16 : mbd[q, (oc,o8p,b)] = (q//16 == o8p)
  msk [128,  512] f32  : msk[q, (o8p,b)]   = (q//16 == o8p) / WS
Output:
  vout [128, 256] f32 : vout[o8*16+j, oc*64+b] = v[b, oc*8+o8, j]
"""

import json
import os
import sys

import numpy as np
import ml_dtypes

sys.path.insert(0, "/opt/trn_rl_repo")

B, IN_CAPS, IN_DIM = 64, 4096, 8
OUT_CAPS, OUT_DIM = 32, 16
EPS = 1e-8
N_CORES = 8
I_LOC = IN_CAPS // N_CORES  # 512
NG = I_LOC // 128           # 4

_BF16 = ml_dtypes.bfloat16

# Engine-assignment plan. Letters: A=ACT(scalar), D=DVE(vector), P=Pool(gpsimd)
PLAN = {
    # one char per wv-phase drain chunk, idx = k*2 + ocp (16 chunks of
    # [128,1024] f32 PSUM -> bf16 SBUF per Gq). Pool/GPSIMD cannot read
    # PSUM (BIR verifier), so only A/D are valid here.
    "drain": "AAAAAAAAAAAAAAAA",
    # drain schedule for the FIRST Gq of each iteration (measured: DVE
    # shares regress — ACT-only wins)
    "drain_g0": "AAAAAAAAAAAAAAAA",
    # which k's (0..7) of the cu build (c*u) go to Pool
    "cu_pool_ks": (5, 6, 7),
    # same, for the LAST Gq: its cu chunks gate s_mm -> exchange, so Pool
    # lag there extends the iteration tail
    "cu_pool_ks_last": (),
    # number of TAIL k's (0 or 2) of the a-path mul (Wv*u) + their pair
    # add that run on Pool instead of DVE
    "amul_pool": 0,
    # squash via fused tensor_scalar pow (vs Ln/Exp on ACT)
    "sq_pow": True,
    # newton refinement steps for the quake rsqrt seed (1 => ~0.17% max err)
    "sq_newton": 1,
    # fp8 payload for the routing AllGathers (iterations 1, 2).
    # Measured on HW: costs ~1.3e-2 of rel err — too close to the 2e-2
    # gate to enable by default.
    "ag_fp8": False,
    # W layouts in fp8e4 (scaled by WS), stationary-only. Measured on HW:
    # ~2.4e-2 rel err through routing feedback — keep off.
    "wfp8": False,
}
if os.environ.get("CAPS_PLAN"):
    PLAN.update(json.loads(os.environ["CAPS_PLAN"]))

WS = 16.0  # host-side W scale for fp8 dynamic range (descaled on chip)

_CACHE = {}


def _build_program(reps=1, variant="full", plan=None):
    import concourse.bass as bass
    import concourse.bacc as bacc
    import concourse.mybir as mybir
    import concourse.tile as tile
    from contextlib import ExitStack

    p = dict(PLAN)
    if plan:
        p.update(plan)

    f32 = mybir.dt.float32
    bf16 = mybir.dt.bfloat16
    fp8 = mybir.dt.float8e4
    wdt = fp8 if p["wfp8"] else bf16
    wscale = WS if p["wfp8"] else 1.0
    vset = set(variant.split(","))
    exch = ("ag" if "ag" in vset else
            "ar" if "ar" in vset else
            "fakeexch" if "fakeexch" in vset else
            "noexch" if "noexch" in vset else "ag")
    AF = mybir.ActivationFunctionType
    ALU = mybir.AluOpType

    nc = bacc.Bacc(
        "TRN2",
        target_bir_lowering=False,
        debug=False,
        enable_asserts=False,
        num_devices=N_CORES,
    )

    wd_d = nc.dram_tensor("wd", [128, 16384], wdt, kind="ExternalInput")
    wb_d = nc.dram_tensor("wb", [128, 16384], wdt, kind="ExternalInput")
    uy_d = nc.dram_tensor("uy", [128, 2048], bf16, kind="ExternalInput")
    dlt_d = nc.dram_tensor("dlt", [128, 128], bf16, kind="ExternalInput")
    mbd_d = nc.dram_tensor("mbd", [128, 2048], bf16, kind="ExternalInput")
    msk_d = nc.dram_tensor("msk", [128, 512], f32, kind="ExternalInput")
    vout_d = nc.dram_tensor("vout", [128, 256], f32, kind="ExternalOutput")

    with tile.TileContext(nc) as tc:
        with ExitStack() as ctx:
            sb = ctx.enter_context(tc.tile_pool(name="sb", bufs=1))
            ps_pool_s = ctx.enter_context(
                tc.tile_pool(name="ps_s", bufs=1, space="PSUM"))
            ps_pool_wv = ctx.enter_context(
                tc.tile_pool(name="ps_wv", bufs=2, space="PSUM"))
            dram = ctx.enter_context(tc.tile_pool(name="dram", bufs=2,
                                                  space="DRAM"))

            WD = sb.tile([128, 16384], wdt, tag="WD")
            WBH = [sb.tile([128, 8192], wdt, tag=f"WB{i}", name=f"WB{i}")
                   for i in range(2)]
            UY = sb.tile([128, 2048], bf16, tag="UY")
            DLT = sb.tile([128, 128], bf16, tag="DLT")
            MBD = sb.tile([128, 2048], bf16, tag="MBD")
            MSK = sb.tile([128, 512], f32, tag="MSK")

            # DMA order: iteration-1 needs WB halves + UY first; WD (a-path)
            # is only needed after the first AllGather; constants in between.
            nc.sync.dma_start(WBH[0][:], wb_d[:, 0:8192])
            nc.sync.dma_start(UY[:, 0:1024], uy_d[:, 0:1024])
            nc.sync.dma_start(WBH[1][:], wb_d[:, 8192:16384])
            nc.sync.dma_start(UY[:, 1024:2048], uy_d[:, 1024:2048])
            nc.sync.dma_start(DLT[:], dlt_d[:])
            nc.sync.dma_start(MBD[:], mbd_d[:])
            nc.sync.dma_start(MSK[:], msk_d[:])
            nc.sync.dma_start(WD[:], wd_d[:])

            def WBs(idx):
                half, off = divmod(idx * 128, 8192)
                return WBH[half][:, off : off + 128]

            # big per-G scratch, parity double-buffered:
            # wv (drains) -> uwv (in-place mul) -> tree partials -> cu
            WVP = [sb.tile([128, 16384], bf16, tag=f"WV{i}", name=f"WV{i}")
                   for i in range(2)]
            BLOG = sb.tile([128, 8192], bf16, tag="BLOG")
            AG = sb.tile([128, 2048], bf16, tag="AG")
            # EG parity-double-buffered so exp(G) overlaps post_b(G-1) reads
            EGP = [sb.tile([128, 2048], bf16, tag=f"EG{i}", name=f"EG{i}")
                   for i in range(2)]
            # URC parity-buffered: Pool's cu(G-1) reads must not block the
            # DVE write of URC(G)
            URCP = [sb.tile([128, 512], bf16, tag=f"URC{i}", name=f"URC{i}")
                    for i in range(2)]
            SMT = sb.tile([128, 1024], bf16, tag="SMT")
            MSKD = sb.tile([128, 512], f32, tag="MSKD")
            GAT = sb.tile([128, 2048], bf16, tag="GAT")
            GAT8 = sb.tile([128, 2048], fp8, tag="GAT8")
            GATB = sb.tile([128, 1024], bf16, tag="GATB")
            SSB = sb.tile([128, 256], f32, tag="SSB")
            SSB1 = sb.tile([128, 256], f32, tag="SSB1")
            SSBH = sb.tile([128, 256], bf16, tag="SSBH")
            SSB8 = sb.tile([128, 256], fp8, tag="SSB8")
            SE = sb.tile([128, 64], f32, tag="SE")
            RCP = sb.tile([128, 64], f32, tag="RCP")
            RCPB = sb.tile([128, 64], bf16, tag="RCPB")
            VBD = sb.tile([128, 2048], bf16, tag="VBD")
            S2 = sb.tile([128, 256], bf16, tag="S2")
            SSP1 = sb.tile([128, 256], f32, tag="SSP1")
            RCP1 = sb.tile([128, 256], f32, tag="RCP1")
            LNV = sb.tile([128, 256], f32, tag="LNV")
            RSQ = sb.tile([128, 256], f32, tag="RSQ")
            T1 = sb.tile([128, 256], f32, tag="T1")
            SCL = sb.tile([128, 256], f32, tag="SCL")
            VSB = sb.tile([128, 256], bf16, tag="VSB")
            VSF = sb.tile([128, 256], f32, tag="VSF")
            MAG = sb.tile([128, 1], mybir.dt.int32, tag="MAG")
            nc.vector.memset(MAG[:], 0x5F3759DF)

            def exchange(t, rep, src):
                """src (partial s) -> cross-core sum back into src."""
                use_fp8 = p["ag_fp8"] and t < 3 and exch == "ag"
                if exch == "fakeexch":
                    # timing probe: same DMAs + adds as "ag", but the
                    # collective is replaced by a local 8x DRAM broadcast
                    nc.vector.tensor_copy(SSBH[:], src[:])
                    ag_in = dram.tile([128, 256], bf16, tag="ag_in",
                                      name=f"ag_in_{rep}_{t}")
                    ag_out = dram.tile([1024, 256], bf16, tag="ag_out",
                                       name=f"ag_out_{rep}_{t}")
                    nc.sync.dma_start(ag_in[:], SSBH[:])
                    nc.sync.dma_start(
                        ag_out[:].rearrange("(r p) f -> r p f", r=8),
                        ag_in[:].unsqueeze(0).broadcast_to([8, 128, 256]),
                    )
                    nc.sync.dma_start(
                        GAT[:].rearrange("p (r f) -> p r f", r=8),
                        ag_out[:].rearrange("(r p) f -> p r f", p=128),
                    )
                    g3 = GAT[:].rearrange("p (r f) -> p r f", r=8)
                    nc.vector.tensor_add(g3[:, 0:4], g3[:, 0:4], g3[:, 4:8])
                    nc.vector.tensor_add(g3[:, 0:2], g3[:, 0:2], g3[:, 2:4])
                    nc.vector.tensor_add(src[:], GAT[:, 0:256],
                                         GAT[:, 256:512])
                    return
                if exch == "ag":
                    if use_fp8:
                        nc.vector.tensor_copy(SSB8[:], src[:])
                        ag_in = dram.tile([128, 256], fp8, tag="ag_in",
                                          name=f"ag_in_{rep}_{t}")
                        ag_out = dram.tile([1024, 256], fp8, tag="ag_out",
                                           name=f"ag_out_{rep}_{t}")
                        nc.sync.dma_start(ag_in[:], SSB8[:])
                    else:
                        nc.vector.tensor_copy(SSBH[:], src[:])
                        ag_in = dram.tile([128, 256], bf16, tag="ag_in",
                                          name=f"ag_in_{rep}_{t}")
                        ag_out = dram.tile([1024, 256], bf16, tag="ag_out",
                                           name=f"ag_out_{rep}_{t}")
                        nc.sync.dma_start(ag_in[:], SSBH[:])
                    nc.gpsimd.collective_compute(
                        "AllGather", ALU.bypass,
                        replica_groups=[list(range(N_CORES))],
                        ins=[ag_in[:].opt()], outs=[ag_out[:].opt()],
                    )
                    if use_fp8:
                        nc.sync.dma_start(
                            GAT8[:].rearrange("p (r f) -> p r f", r=8),
                            ag_out[:].rearrange("(r p) f -> p r f", p=128),
                        )
                        nc.vector.tensor_add(GATB[:], GAT8[:, 0:1024],
                                             GAT8[:, 1024:2048])
                        nc.vector.tensor_add(GATB[:, 0:512], GATB[:, 0:512],
                                             GATB[:, 512:1024])
                        nc.vector.tensor_add(src[:], GATB[:, 0:256],
                                             GATB[:, 256:512])
                    else:
                        nc.sync.dma_start(
                            GAT[:].rearrange("p (r f) -> p r f", r=8),
                            ag_out[:].rearrange("(r p) f -> p r f", p=128),
                        )
                        g3 = GAT[:].rearrange("p (r f) -> p r f", r=8)
                        nc.vector.tensor_add(g3[:, 0:4], g3[:, 0:4], g3[:, 4:8])
                        nc.vector.tensor_add(g3[:, 0:2], g3[:, 0:2], g3[:, 2:4])
                        nc.vector.tensor_add(src[:], GAT[:, 0:256],
                                             GAT[:, 256:512])
                elif exch == "ar":
                    ar_in = dram.tile([128, 256], f32, tag="ar_in",
                                      name=f"ar_in_{rep}_{t}")
                    ar_out = dram.tile([128, 256], f32, tag="ar_out",
                                       name=f"ar_out_{rep}_{t}")
                    nc.gpsimd.dma_start(ar_in[:], src[:])
                    nc.gpsimd.collective_compute(
                        "AllReduce", ALU.add,
                        replica_groups=[list(range(N_CORES))],
                        ins=[ar_in[:].opt()], outs=[ar_out[:].opt()],
                    )
                    nc.gpsimd.dma_start(src[:], ar_out[:])
                elif exch == "noexch":
                    pass  # timing-only: skip the cross-core exchange

            def squash(t, rep, src=None):
                src = SSB if src is None else src
                exchange(t, rep, src)
                # ss = sum_j s^2 (dup'd over 16 via DLT matmul)
                nc.vector.tensor_mul(S2[:], src[:], src[:])
                ps_sq = ps_pool_wv.tile([128, 256], f32, tag="wvp",
                                        name=f"ps_sq_{rep}_{t}")
                nc.tensor.matmul(ps_sq[:], DLT[:], S2[:], start=True, stop=True)
                if p["sq_pow"]:
                    # scale = (ss/(1+ss)) * rsqrt(ss+eps), all on DVE.
                    # rsqrt via quake seed + 2 Newton steps (no ACT tables).
                    i32 = mybir.dt.int32
                    nc.vector.tensor_scalar(
                        T1[:].bitcast(i32), ps_sq[:].bitcast(i32), 1, None,
                        op0=ALU.logical_shift_right)
                    nc.vector.tensor_tensor(
                        RSQ[:].bitcast(i32), MAG[:].broadcast_to([128, 256]),
                        T1[:].bitcast(i32), op=ALU.subtract)
                    nc.vector.tensor_scalar(LNV[:], ps_sq[:], -0.5,
                                            -0.5 * EPS, op0=ALU.mult,
                                            op1=ALU.add)
                    for _ in range(p["sq_newton"]):
                        nc.vector.tensor_mul(T1[:], RSQ[:], RSQ[:])
                        nc.vector.tensor_mul(T1[:], LNV[:], T1[:])
                        nc.vector.tensor_scalar_add(T1[:], T1[:], 1.5)
                        nc.vector.tensor_mul(RSQ[:], RSQ[:], T1[:])
                    nc.vector.tensor_scalar_add(SSP1[:], ps_sq[:], 1.0)
                    nc.vector.reciprocal(RCP1[:], SSP1[:])
                    nc.vector.tensor_mul(T1[:], ps_sq[:], RCP1[:])
                    nc.vector.tensor_mul(SCL[:], T1[:], RSQ[:])
                else:
                    # rsqrt via exp(-0.5*ln)
                    nc.vector.tensor_scalar_add(SSP1[:], ps_sq[:], 1.0)
                    nc.vector.reciprocal(RCP1[:], SSP1[:])
                    nc.vector.tensor_scalar_add(LNV[:], ps_sq[:], EPS)
                    nc.scalar.activation(LNV[:], LNV[:], AF.Ln)
                    nc.scalar.activation(RSQ[:], LNV[:], AF.Exp, scale=-0.5)
                    nc.vector.tensor_mul(T1[:], RCP1[:], RSQ[:])
                    nc.vector.tensor_mul(SCL[:], ps_sq[:], T1[:])
                if t < 3:
                    nc.vector.tensor_mul(VSB[:], src[:], SCL[:])
                    vsb_b = (VSB[:].rearrange("q (oc b) -> q oc b", oc=4)
                             .unsqueeze(2).broadcast_to([128, 4, 8, 64]))
                    mbd4 = MBD[:].rearrange("q (oc o8 b) -> q oc o8 b",
                                            oc=4, o8=8)
                    vbd4 = VBD[:].rearrange("q (oc o8 b) -> q oc o8 b",
                                            oc=4, o8=8)
                    nc.vector.tensor_mul(vbd4, vsb_b, mbd4)
                else:
                    nc.vector.tensor_mul(VSF[:], src[:], SCL[:])
                    nc.sync.dma_start(vout_d[:], VSF[:])

            def drain(eng_ch, dst, src):
                if wscale != 1.0:
                    if eng_ch == "A":
                        nc.scalar.mul(dst, src, 1.0 / wscale)
                    elif eng_ch == "D":
                        nc.vector.tensor_scalar_mul(dst, src, 1.0 / wscale)
                    else:
                        nc.gpsimd.tensor_scalar_mul(dst, src, 1.0 / wscale)
                else:
                    if eng_ch == "A":
                        nc.scalar.copy(dst, src)
                    elif eng_ch == "D":
                        nc.vector.tensor_copy(dst, src)
                    else:
                        nc.gpsimd.tensor_copy(dst, src)

            def eng_of(ch):
                return nc.vector if ch == "D" else nc.gpsimd

            def issue_s1(rep):
                """iteration 1: s1 = (1/32) sum_i u_hat, extracted to SSB1.

                Issued BEFORE the previous rep's squash(3) so the 128
                matmuls fill the PE during that exchange. PSUM comes from
                the wv pool (free at the prior rep's tail); each oc
                accumulation group gets its own PSUM bank (start=True
                resets per-bank): oc pairs at offsets 0/512 of two
                [128,1024] (2-bank) tiles.
                """
                ps_s1 = [ps_pool_wv.tile([128, 1024], f32, tag="wvp",
                                         name=f"ps_s0_{rep}_{h}")
                         for h in range(2)]

                def s1_slot(oc):
                    return ps_s1[oc // 2][:, (oc % 2) * 512 :
                                          (oc % 2) * 512 + 64]
                for Gq in range(NG):
                    for k in range(8):
                        for oc in range(4):
                            nc.tensor.matmul(
                                s1_slot(oc),
                                WBs((Gq * 8 + k) * 4 + oc),
                                UY[:, (Gq * 8 + k) * 64 :
                                      (Gq * 8 + k) * 64 + 64],
                                start=(Gq == 0 and k == 0),
                                stop=(Gq == NG - 1 and k == 7),
                            )
                for oc in range(4):
                    nc.scalar.mul(SSB1[:, oc * 64 : oc * 64 + 64],
                                  s1_slot(oc), 1.0 / (32.0 * wscale))

            for rep in range(reps):
                if rep == 0:
                    issue_s1(0)
                squash(1, rep, src=SSB1)

                # ---------- iterations 2, 3 ----------
                for t in (2, 3):
                    ps_s = ps_pool_s.tile([128, 2048], f32, tag="ps_s",
                                          name=f"ps_s{rep}_{t}")

                    def wv_half(Gq, ks):
                        WVG = WVP[Gq % 2]
                        for k in ks:
                            for ocp in range(2) if "skipwv" not in vset else []:
                                wvp = ps_pool_wv.tile(
                                    [128, 1024], f32, tag="wvp",
                                    name=f"wvp_{rep}_{t}_{Gq}_{k}_{ocp}")
                                for kk in range(2):
                                    oc = ocp * 2 + kk
                                    nc.tensor.matmul(
                                        wvp[:, kk * 512 : kk * 512 + 512],
                                        WD[:, ((oc * 4 + Gq) * 8 + k) * 128 :
                                              ((oc * 4 + Gq) * 8 + k) * 128
                                              + 128],
                                        VBD[:, oc * 512 : oc * 512 + 512],
                                        start=True, stop=True,
                                    )
                                dst = WVG[:, k * 2048 + ocp * 1024 :
                                             k * 2048 + ocp * 1024 + 1024]
                                dplan = (p["drain_g0"] if Gq == 0
                                         else p["drain"])
                                drain(dplan[k * 2 + ocp], dst, wvp[:])

                    def post_a(Gq):
                        WVG = WVP[Gq % 2]
                        EG = EGP[Gq % 2]
                        wvg4 = WVG[:].rearrange("p (k x b) -> p k x b",
                                                k=8, x=32)
                        uyg4 = (UY[:, Gq * 512 : Gq * 512 + 512]
                                .rearrange("p (k b) -> p k b", k=8)
                                .unsqueeze(2).broadcast_to([128, 8, 32, 64]))
                        # k-slab pair view for the strided halving tree:
                        # [p, kp, two, x] with two = adjacent k-slabs
                        wpair = WVG[:].rearrange(
                            "p (kp two x) -> p kp two x", two=2, x=2048)
                        npk = p["amul_pool"]  # 0 or 2 tail ks on Pool
                        dk = 8 - npk
                        if "skipmt" not in vset:
                            # Pool takes the tail ks (slow engine, issued
                            # first so it overlaps the DVE halves)
                            if npk:
                                nc.gpsimd.tensor_mul(
                                    wvg4[:, dk:8], wvg4[:, dk:8],
                                    uyg4[:, dk:8])
                                nc.gpsimd.tensor_add(
                                    wpair[:, dk // 2 : 4, 0],
                                    wpair[:, dk // 2 : 4, 0],
                                    wpair[:, dk // 2 : 4, 1])
                            # a-path mul in two k-halves (chases the drains),
                            # paired k-sum tree right behind each half
                            nc.vector.tensor_mul(
                                wvg4[:, 0:4], wvg4[:, 0:4], uyg4[:, 0:4])
                            nc.vector.tensor_add(
                                wpair[:, 0:2, 0], wpair[:, 0:2, 0],
                                wpair[:, 0:2, 1])
                            if dk > 4:
                                nc.vector.tensor_mul(
                                    wvg4[:, 4:dk], wvg4[:, 4:dk],
                                    uyg4[:, 4:dk])
                                nc.vector.tensor_add(
                                    wpair[:, 2 : dk // 2, 0],
                                    wpair[:, 2 : dk // 2, 0],
                                    wpair[:, 2 : dk // 2, 1])
                            # quads: (k01)+=(k23) at slabs 0,8192; +4096 src
                            wq = WVG[:].rearrange(
                                "p (kq rest) -> p kq rest", kq=2)
                            nc.vector.tensor_add(
                                wq[:, :, 0:2048], wq[:, :, 0:2048],
                                wq[:, :, 4096:6144])
                        gsl = slice(Gq * 2048, Gq * 2048 + 2048)
                        if t == 2:
                            # final tree add lands directly in BLOG
                            nc.vector.tensor_add(
                                BLOG[:, gsl], WVG[:, 0:2048],
                                WVG[:, 8192:10240])
                            nc.scalar.activation(EG[:], BLOG[:, gsl], AF.Exp)
                        else:
                            nc.vector.tensor_add(
                                AG[:], WVG[:, 0:2048], WVG[:, 8192:10240])
                            nc.vector.tensor_add(AG[:], BLOG[:, gsl], AG[:])
                            nc.scalar.activation(EG[:], AG[:], AF.Exp)

                    def post_b(Gq):
                        WVG = WVP[Gq % 2]
                        EG = EGP[Gq % 2]
                        URC = URCP[Gq % 2]
                        wvg4 = WVG[:].rearrange("p (k x b) -> p k x b",
                                                k=8, x=32)
                        # sum over o: contiguous halving tree (o is outer)
                        nc.vector.tensor_add(SMT[:], EG[:, 0:1024],
                                             EG[:, 1024:2048])
                        nc.vector.tensor_add(SMT[:, 0:512], SMT[:, 0:512],
                                             SMT[:, 512:1024])
                        nc.vector.tensor_add(SMT[:, 0:256], SMT[:, 0:256],
                                             SMT[:, 256:512])
                        nc.vector.tensor_add(SMT[:, 0:128], SMT[:, 0:128],
                                             SMT[:, 128:256])
                        nc.vector.tensor_add(SE[:], SMT[:, 0:64],
                                             SMT[:, 64:128])
                        nc.vector.reciprocal(RCP[:], SE[:])
                        # urc = u * (1/Z): folds softmax denom into cu
                        # (RCP stays f32; avoids an ACT round-trip)
                        nc.vector.tensor_mul(
                            URC[:].rearrange("p (k b) -> p k b", k=8),
                            UY[:, Gq * 512 : Gq * 512 + 512]
                               .rearrange("p (k b) -> p k b", k=8),
                            RCP[:].unsqueeze(1).broadcast_to([128, 8, 64]))
                        # cu = e * urc (into WVG, now dead), built per-k so
                        # the s-path matmuls chase each chunk on PE
                        egb4 = (EG[:].rearrange("p (x b) -> p x b", x=32)
                                .unsqueeze(1).broadcast_to([128, 8, 32, 64]))
                        urc4 = (URC[:].rearrange("p (k b) -> p k b", k=8)
                                .unsqueeze(2).broadcast_to([128, 8, 32, 64]))
                        cks = set(p["cu_pool_ks"] if Gq < NG - 1
                                  else p["cu_pool_ks_last"])
                        # Pool chunks issued first (slow engine, runs in
                        # parallel with DVE's chunks)
                        if "skipcu" not in vset:
                            for k in sorted(cks):
                                nc.gpsimd.tensor_mul(
                                    wvg4[:, k:k+1], egb4[:, k:k+1],
                                    urc4[:, k:k+1])
                        def s_mm(k):
                            for oc in range(4):
                                nc.tensor.matmul(
                                    ps_s[:, oc * 512 : oc * 512 + 512],
                                    WBs((Gq * 8 + k) * 4 + oc),
                                    WVG[:, k * 2048 + oc * 512 :
                                           k * 2048 + oc * 512 + 512],
                                    start=(Gq == 0 and k == 0),
                                    stop=(Gq == NG - 1 and k == 7),
                                )
                        dks = [k for k in range(8) if k not in cks]
                        for k in dks:
                            if "skipcu" not in vset:
                                nc.vector.tensor_mul(
                                    wvg4[:, k:k+1], egb4[:, k:k+1],
                                    urc4[:, k:k+1])
                            if "skipsmm" not in vset:
                                s_mm(k)
                        if "skipsmm" not in vset:
                            for k in sorted(cks):
                                s_mm(k)

                    # software pipeline: wv(G) is split in halves with
                    # post_a(G-1) issued between them so exp(G-1) sits
                    # behind only 8 drains in the ACT FIFO, not 16.
                    for Gq in range(NG + 1):
                        if Gq < NG:
                            wv_half(Gq, range(0, 4))
                        if Gq >= 1:
                            post_a(Gq - 1)
                        if Gq < NG:
                            wv_half(Gq, range(4, 8))
                        if Gq >= 1:
                            post_b(Gq - 1)
                    # fused diag extract via mask
                    if "skipsmm" not in vset:
                        for oc in range(4):
                            nc.vector.tensor_mul(
                                MSKD[:], ps_s[:, oc * 512 : oc * 512 + 512],
                                MSK[:])
                            nc.vector.tensor_reduce(
                                SSB[:, oc * 64 : oc * 64 + 64],
                                MSKD[:].rearrange("q (o8 b) -> q b o8", o8=8),
                                axis=mybir.AxisListType.X, op=ALU.add)
                    # next rep's iteration-1 matmuls fill the PE during
                    # this rep's final exchange/squash
                    if t == 3 and rep + 1 < reps:
                        issue_s1(rep + 1)
                    squash(t, rep)

    nc.compile()
    return nc


def _host_prep(u, W, plan=None):
    """Build per-core input maps (all host-side permutes)."""
    import concourse.mybir as mybir

    p = dict(PLAN)
    if plan:
        p.update(plan)
    wnp = mybir.dt.np(mybir.dt.float8e4) if p["wfp8"] else _BF16
    wscale = WS if p["wfp8"] else 1.0

    in_maps = []
    q = np.arange(128)
    dlt = (q[:, None] // 16 == q[None, :] // 16).astype(_BF16)
    o8p = np.arange(8)
    diag = (q[:, None] // 16 == o8p[None, :])
    mbd = np.ascontiguousarray(
        np.broadcast_to(diag[:, None, :, None], (128, 4, 8, 64))
    ).reshape(128, 2048).astype(_BF16)
    msk = np.ascontiguousarray(
        np.broadcast_to(diag[:, :, None], (128, 8, 64))
    ).reshape(128, 512).astype(np.float32) / wscale
    for c in range(N_CORES):
        Ws = np.asarray(W[c * I_LOC : (c + 1) * I_LOC], dtype=np.float32)
        Ws = Ws * wscale
        us = np.asarray(u[:, c * I_LOC : (c + 1) * I_LOC, :], dtype=np.float32)
        Wr = Ws.reshape(NG, 128, 4, 8, 16, 8)           # [Gq,p,oc,o8,j,k]
        wd = np.ascontiguousarray(
            Wr.transpose(3, 4, 2, 0, 5, 1)              # [o8,j,oc,Gq,k,p]
        ).reshape(128, 16384).astype(wnp)
        wb = np.ascontiguousarray(
            Wr.transpose(1, 0, 5, 2, 3, 4)              # [p,Gq,k,oc,o8,j]
        ).reshape(128, 16384).astype(wnp)
        ur = us.reshape(B, NG, 128, 8)                  # [b,Gq,p,k]
        uy = np.ascontiguousarray(
            ur.transpose(2, 1, 3, 0)                    # [p,Gq,k,b]
        ).reshape(128, 2048).astype(_BF16)
        in_maps.append({"wd": wd, "wb": wb, "uy": uy, "dlt": dlt,
                        "mbd": mbd, "msk": msk})
    return in_maps


def kernel(u, W):
    from concourse.bass_utils import run_bass_kernel_spmd

    if "nc" not in _CACHE:
        _CACHE["nc"] = _build_program(variant="ag")
    nc = _CACHE["nc"]

    in_maps = _host_prep(u, W)
    res = run_bass_kernel_spmd(
        nc, in_maps, core_ids=list(range(N_CORES)),
        trace=bool(int(os.environ.get("CAPS_TRACE", "0"))),
    )
    if isinstance(res, tuple):
        results = res[0]
    else:
        _CACHE["last_results"] = res
        results = res.results
    vout = results[0]["vout"]  # [128, 256]; identical on every core
    t = vout.reshape(8, 16, 4, 64)          # [o8, j, oc, b]
    v = np.ascontiguousarray(t.transpose(3, 2, 0, 1)).reshape(B, OUT_CAPS, OUT_DIM)
    return v.astype(np.float32)

